# revision 23
# baseline (speedup 1.0000x reference)
"""A2Net_Cond Trainium2 kernel: 8-core data-parallel over batch.

Per core (one batch element), bf16 compute / f32 accumulate:
  x [C=128, THW=16384] f32 -> bf16.
  Super-chunk loop (4 f-chunks of 128 per iteration):
    conv matmuls (x_c stationary bf16): psum [128f, 4, 192] = [A^T | B0^T | V0^T]
    eb = exp(B0^T) bf16 [f, n]          (theta bias dropped: softmax row-invariant)
    ev = exp(V0^T) * exp(b_rho) f32     (rho bias folded multiplicatively)
    colsum+recip per-partition f; V^T = ev * recip -> TBs[:, :, 0:64] bf16
    attnT = eb * V^T                    -> TBs[:, :, 64:128] bf16
    PE-transpose TBs -> [V ; attnT] [n, f] psum bf16
    evac V -> vsb bf16; evac attnT -> attnp bf16 at permuted (hw,t) positions
    A-evac * Cm -> Urhs[:, :, 0:64]; [Cm|1] ext -> Urhs[:, :, 64:66]
    U psum [n, 66] += eb.T @ Urhs      (cols: U0^T | rowsumBC | rowsumB)
  Post: AB_T^T = (U0^T + rowsumBC x b_phi) / rowsumB;  Z = AB_T^T.T @ V (bf16 out)
        attn = attnp * (1/rowsumB) (bf16 out); host converts outputs to f32.
"""

import os
import sys

import numpy as np

for _p in ("/opt/trn_rl_repo", "/root/.axon_site/_ro/trn_rl_repo"):
    if _p not in sys.path and os.path.isdir(_p):
        sys.path.append(_p)

B, C, T, H, W = 8, 128, 16, 32, 32
M, N, CS = 64, 64, 64
THW = T * H * W  # 16384
NCORES = 8

FCHUNK = 128
NCHUNK = THW // FCHUNK  # 128
HWBLK = (H * W) // FCHUNK  # 8 chunks per t-plane
CB = 4  # chunks per super-chunk
NSC = NCHUNK // CB  # 32

_NC_CACHE = {}


def build_nc():
    import concourse.bacc as bacc
    import concourse.bass as bass
    import concourse.mybir as mybir
    import concourse.tile as tile
    from concourse.masks import make_identity

    fp32 = mybir.dt.float32
    bf16 = mybir.dt.bfloat16
    AF = mybir.ActivationFunctionType

    nc = bacc.Bacc(None, target_bir_lowering=False)

    x_d = nc.declare_dram_parameter("x", [C, THW], mybir.dt.float32r, isOutputFalse=False) if False else nc.declare_dram_parameter("x", [C, THW], mybir.dt.float32r, isOutput=False)
    f32r = mybir.dt.float32r
    wt_d = nc.declare_dram_parameter("wt_cat", [C, 4 * M], f32r, isOutput=False)
    w1t_d = nc.declare_dram_parameter("w1t", [CS, CS], fp32, isOutput=False)
    b1_d = nc.declare_dram_parameter("b1", [CS, 1], fp32, isOutput=False)
    w2t_d = nc.declare_dram_parameter("w2t", [CS, H * W], fp32, isOutput=False)
    b2pt_d = nc.declare_dram_parameter("b2pt", [FCHUNK, HWBLK], fp32, isOutput=False)
    cond_d = nc.declare_dram_parameter("cond", [CS, 1], fp32, isOutput=False)
    ebr_d = nc.declare_dram_parameter("ebrho_b", [FCHUNK, N], bf16, isOutput=False)
    bphi_d = nc.declare_dram_parameter("bphi_b", [N, M], fp32, isOutput=False)
    ebrc_d = nc.declare_dram_parameter("ebrho_col", [N, 1], fp32, isOutput=False)
    oz_d = nc.declare_dram_parameter("out_z", [M, THW], bf16, isOutput=True)
    oa_d = nc.declare_dram_parameter("out_attn", [N, THW], bf16, isOutput=True)

    with tile.TileContext(nc) as tc:
        import contextlib

        ctx = contextlib.ExitStack()
        with ctx:
            singles = ctx.enter_context(tc.tile_pool(name="singles", bufs=1))
            xpool = ctx.enter_context(tc.tile_pool(name="xp", bufs=3))
            work = ctx.enter_context(tc.tile_pool(name="work", bufs=3))
            psum_c = ctx.enter_context(tc.tile_pool(name="ps_c", bufs=2, space="PSUM"))
            psum_t = ctx.enter_context(tc.tile_pool(name="ps_t", bufs=2, space="PSUM"))
            psum_u = ctx.enter_context(tc.tile_pool(name="ps_u", bufs=1, space="PSUM"))
            psum_z = ctx.enter_context(tc.tile_pool(name="ps_z", bufs=1, space="PSUM"))

            ident = singles.tile([FCHUNK, FCHUNK], bf16)
            make_identity(nc, ident)

            wt_sb = singles.tile([C, 4 * M], f32r)
            nc.sync.dma_start(out=wt_sb, in_=wt_d[:, :])
            w1t_sb = singles.tile([CS, CS], fp32)
            nc.sync.dma_start(out=w1t_sb, in_=w1t_d[:, :])
            b1_sb = singles.tile([CS, 1], fp32)
            nc.sync.dma_start(out=b1_sb, in_=b1_d[:, :])
            w2t_sb = singles.tile([CS, H * W], fp32)
            nc.sync.dma_start(out=w2t_sb, in_=w2t_d[:, :])
            b2pt_sb = singles.tile([FCHUNK, HWBLK], fp32)
            nc.sync.dma_start(out=b2pt_sb, in_=b2pt_d[:, :])
            cond_sb = singles.tile([CS, 1], fp32)
            nc.sync.dma_start(out=cond_sb, in_=cond_d[:, :])
            ebr_sb = singles.tile([FCHUNK, 1, N], bf16)
            nc.sync.dma_start(out=ebr_sb, in_=ebr_d[:, :].rearrange("p (o n) -> p o n", o=1))
            bphi_sb = singles.tile([N, M], fp32)
            nc.sync.dma_start(out=bphi_sb, in_=bphi_d[:, :])
            ebrc_sb = singles.tile([N, 1], fp32)
            nc.sync.dma_start(out=ebrc_sb, in_=ebrc_d[:, :])

            ones_col = singles.tile([C, 1], fp32)
            nc.vector.memset(ones_col, 1.0)
            ones_row = singles.tile([1, C], fp32)
            nc.vector.memset(ones_row, 1.0)

            # ---------------- tiny FC-cond (fp32, negligible) ----------------
            ps_h1 = psum_t.tile([CS, 1], fp32, tag="pt")
            nc.tensor.matmul(ps_h1, w1t_sb, cond_sb)
            h1_sb = work.tile([CS, 1], fp32)
            nc.scalar.activation(h1_sb, ps_h1, AF.Relu, bias=b1_sb, scale=1.0)
            ps_h2 = psum_t.tile([FCHUNK, HWBLK], fp32, tag="pt")
            for k in range(HWBLK):
                nc.tensor.matmul(
                    ps_h2[:, k : k + 1],
                    w2t_sb[:, k * FCHUNK : (k + 1) * FCHUNK],
                    h1_sb,
                )
            h2_sb = work.tile([FCHUNK, HWBLK], fp32)
            nc.vector.tensor_add(h2_sb, ps_h2, b2pt_sb)
            nc.scalar.activation(h2_sb, h2_sb, AF.Relu)
            expc_sb = work.tile([FCHUNK, HWBLK], fp32)
            sc_sb = work.tile([FCHUNK, 1], fp32)
            nc.scalar.activation(expc_sb, h2_sb, AF.Exp, accum_out=sc_sb)
            ps_s = psum_t.tile([1, 1], fp32, tag="pt")
            nc.tensor.matmul(ps_s, ones_col, sc_sb)
            rc_sb = work.tile([1, 1], fp32)
            nc.vector.reciprocal(rc_sb, ps_s)
            ps_rc = psum_t.tile([FCHUNK, 1], fp32, tag="pt")
            nc.tensor.matmul(ps_rc, ones_row, rc_sb)
            rcp_sb = work.tile([FCHUNK, 1], fp32)
            nc.scalar.copy(rcp_sb, ps_rc)
            cm_sb = singles.tile([FCHUNK, HWBLK, 1], fp32)
            nc.vector.tensor_scalar_mul(
                cm_sb, in0=expc_sb.rearrange("p (k o) -> p k o", o=1), scalar1=rcp_sb
            )
            cmones_sb = singles.tile([FCHUNK, HWBLK, 2], bf16)
            nc.vector.memset(cmones_sb, 1.0)
            nc.vector.tensor_copy(cmones_sb[:, :, 0:1], cm_sb)

            # ---------------- x load (f32 resident; convs use f32r) ----------------
            XPIECE = 2048
            NXD = THW // XPIECE  # 8
            x_sb = singles.tile([C, THW], f32r)
            for i in range(NXD):
                s = i * XPIECE
                nc.sync.dma_start(out=x_sb[:, s : s + XPIECE], in_=x_d[:, s : s + XPIECE])

            # ---------------- main loop ----------------
            vsb = singles.tile([N, THW], bf16)
            attq = singles.tile([N, THW], bf16)  # attn product, t-major
            ps_u_t = psum_u.tile([N, M + 2], fp32)

            for sc in range(NSC):
                c0 = sc * CB
                f0 = c0 * FCHUNK
                tidx = c0 // HWBLK
                hwb0 = c0 % HWBLK  # 0 or 4

                pc = psum_c.tile([FCHUNK, CB, 4 * M], fp32, tag="pc")
                for j in range(CB):
                    nc.tensor.matmul(
                        pc[:, j, :],
                        x_sb[:, f0 + j * FCHUNK : f0 + (j + 1) * FCHUNK],
                        wt_sb,
                    )

                ebv = work.tile([FCHUNK, CB, 2 * N], bf16, tag="ebv")
                nc.scalar.activation(ebv, pc[:, :, M : 3 * M], AF.Exp)
                eb = ebv[:, :, 0:N]
                ev = ebv[:, :, N : 2 * N]
                csum = work.tile([FCHUNK, CB], fp32, tag="csum")
                scr = work.tile([FCHUNK, CB, N], fp32, tag="scr")
                nc.vector.tensor_mul(scr, ev, ebr_sb.to_broadcast([FCHUNK, CB, N]))
                nc.vector.reduce_sum(csum, scr, axis=mybir.AxisListType.X)
                nc.vector.reciprocal(csum, csum)
                tbs = work.tile([FCHUNK, CB, 2 * N], bf16, tag="tbs")
                nc.vector.tensor_mul(
                    tbs[:, :, 0:N],
                    ev,
                    csum.rearrange("p (c o) -> p c o", o=1).to_broadcast(
                        [FCHUNK, CB, N]
                    ),
                )
                nc.vector.tensor_mul(tbs[:, :, N : 2 * N], eb, tbs[:, :, 0:N])

                pt = psum_t.tile([FCHUNK, CB, FCHUNK], bf16, tag="pt")
                for j in range(CB):
                    nc.tensor.transpose(pt[:, j, :], tbs[:, j, :], ident)

                nc.vector.tensor_copy(vsb[:, f0 : f0 + CB * FCHUNK], pt[0:N, :, :])
                nc.scalar.copy(attq[:, f0 : f0 + CB * FCHUNK], pt[N : 2 * N, :, :])

                urhs = work.tile([FCHUNK, CB, M + 2], bf16, tag="urhs")
                nc.vector.tensor_mul(
                    urhs[:, :, 0:M],
                    pc[:, :, 0:M],
                    cm_sb[:, hwb0 : hwb0 + CB, :].to_broadcast([FCHUNK, CB, M]),
                )
                nc.vector.tensor_copy(
                    urhs[:, :, M : M + 2], cmones_sb[:, hwb0 : hwb0 + CB, :]
                )
                for j in range(CB):
                    nc.tensor.matmul(
                        ps_u_t,
                        ebv[:, j, 0:N],
                        urhs[:, j, :],
                        start=(sc == 0 and j == 0),
                        stop=(sc == NSC - 1 and j == CB - 1),
                    )

            # ---------------- post loop ----------------
            u_sb = work.tile([N, M + 2], fp32)
            nc.scalar.copy(u_sb, ps_u_t)
            rb_sb = work.tile([N, 1], fp32)
            nc.vector.reciprocal(rb_sb, u_sb[:, M + 1 : M + 2])
            corr = work.tile([N, M], fp32)
            nc.vector.tensor_scalar_mul(corr, in0=bphi_sb, scalar1=u_sb[:, M : M + 1])
            nc.vector.tensor_add(corr, corr, u_sb[:, 0:M])
            rbw_sb = work.tile([N, 1], fp32)
            nc.vector.tensor_mul(rbw_sb, rb_sb, ebrc_sb)
            abt_sb = work.tile([N, M], bf16)
            nc.vector.tensor_scalar_mul(abt_sb, in0=corr, scalar1=rbw_sb)

            ZBLK = 512
            for bi in range(THW // ZBLK):
                s = bi * ZBLK
                pz = psum_z.tile([M, ZBLK], fp32, tag="pz")
                nc.tensor.matmul(pz, abt_sb, vsb[:, s : s + ZBLK])
                zt = work.tile([M, ZBLK], bf16, tag="zt")
                nc.scalar.copy(zt, pz)
                nc.sync.dma_start(out=oz_d[:, s : s + ZBLK], in_=zt)

            attq_v = attq.rearrange("p (t hw) -> p hw t", t=T)
            hhw = (H * W) // 2
            for ah in range(2):
                attnf = work.tile([N, THW // 2], bf16, tag="attnf")
                if ah == 0:
                    nc.vector.tensor_scalar_mul(
                        attnf.rearrange("p (hw t) -> p hw t", t=T),
                        in0=attq_v[:, ah * hhw : (ah + 1) * hhw, :],
                        scalar1=rbw_sb,
                    )
                else:
                    nc.scalar.mul(
                        attnf.rearrange("p (hw t) -> p hw t", t=T),
                        attq_v[:, ah * hhw : (ah + 1) * hhw, :],
                        rbw_sb,
                    )
                nc.sync.dma_start(
                    out=oa_d[:, ah * (THW // 2) : (ah + 1) * (THW // 2)], in_=attnf
                )


    nc.compile()
    return nc


def _prep_shared(W_phi, b_phi, W_theta, b_theta, W_rho, b_rho, W1, b1, W2, b2):
    import ml_dtypes

    f32 = np.float32
    bf = ml_dtypes.bfloat16
    wt_cat = np.concatenate(
        [np.asarray(W_phi).T, np.asarray(W_theta).T, np.asarray(W_rho).T], axis=1
    ).astype(f32)
    shared = {
        "wt_cat": np.ascontiguousarray(
            np.concatenate([wt_cat, np.zeros((C, M), f32)], axis=1)
        ),
        "w1t": np.ascontiguousarray(np.asarray(W1).T.astype(f32)),
        "b1": np.asarray(b1).astype(f32).reshape(CS, 1),
        "w2t": np.ascontiguousarray(np.asarray(W2).T.astype(f32)),
        "b2pt": np.ascontiguousarray(
            np.asarray(b2).astype(f32).reshape(HWBLK, FCHUNK).T
        ),
        "ebrho_b": np.ascontiguousarray(
            np.tile(np.exp(np.asarray(b_rho).astype(f32))[None, :], (FCHUNK, 1)).astype(bf)
        ),
        "bphi_b": np.ascontiguousarray(
            np.tile(np.asarray(b_phi).astype(f32)[None, :], (N, 1))
        ),
        "ebrho_col": np.ascontiguousarray(
            np.exp(np.asarray(b_rho).astype(f32)).reshape(N, 1)
        ),
    }
    return shared


def _install_ntff_hook():
    """The container's antenv stub lacks axon_hooks; inject it and register
    the ctypes NTFF profiling hook so trace=True works under axon."""
    import sys
    import types

    if "antenv.axon_hooks" in sys.modules:
        return
    import antenv

    mod = types.ModuleType("antenv.axon_hooks")
    _state = {}
    mod.set_axon_ntff_profile_hook = lambda h: _state.__setitem__("h", h)
    mod.get_axon_ntff_profile_hook = lambda: _state.get("h")
    sys.modules["antenv.axon_hooks"] = mod
    antenv.axon_hooks = mod
    try:
        from trn_agent_boot.trn_boot import _ntff_profile_via_ctypes

        hook = _ntff_profile_via_ctypes("/opt/axon/libaxon_pjrt.so")
        if hook is not None:
            mod.set_axon_ntff_profile_hook(hook)
    except Exception as e:  # profiling degrades, run still works
        print("ntff hook install failed:", e)


def run_cores(inputs, trace=False):
    from concourse.bass_utils import run_bass_kernel_spmd

    if trace:
        _install_ntff_hook()

    if "nc" not in _NC_CACHE:
        _NC_CACHE["nc"] = build_nc()
    nc = _NC_CACHE["nc"]

    x_full = np.asarray(inputs["input"], dtype=np.float32).reshape(B, C, THW)
    cond_full = np.asarray(inputs["condition"], dtype=np.float32)
    shared = _prep_shared(
        inputs["W_phi"], inputs["b_phi"], inputs["W_theta"], inputs["b_theta"],
        inputs["W_rho"], inputs["b_rho"], inputs["W1"], inputs["b1"],
        inputs["W2"], inputs["b2"],
    )
    in_maps = []
    for b in range(NCORES):
        m = dict(shared)
        m["x"] = np.ascontiguousarray(x_full[b])
        m["cond"] = np.ascontiguousarray(cond_full[b].reshape(CS, 1))
        in_maps.append(m)

    kw = {}
    if trace:
        kw = dict(trace=True, trace_cores=[0])
    res = run_bass_kernel_spmd(nc, in_maps, core_ids=list(range(NCORES)), **kw)
    return res


def kernel(**inputs):
    res = run_cores(inputs, trace=False)
    z = np.stack(
        [np.asarray(r["out_z"], dtype=np.float32) for r in res.results], axis=0
    ).reshape(B, M, T, H, W)
    attn = np.stack(
        [np.asarray(r["out_attn"], dtype=np.float32) for r in res.results], axis=0
    ).reshape(B, N, H, W, T)
    return z, attn


# revision 25
# speedup vs baseline: 1.1269x; 1.1269x over previous
"""A2Net_Cond Trainium2 kernel: 8-core data-parallel over batch.

Per core (one batch element), bf16 compute / f32 accumulate:
  x [C=128, THW=16384] f32 -> bf16.
  Super-chunk loop (4 f-chunks of 128 per iteration):
    conv matmuls (x_c stationary bf16): psum [128f, 4, 192] = [A^T | B0^T | V0^T]
    eb = exp(B0^T) bf16 [f, n]          (theta bias dropped: softmax row-invariant)
    ev = exp(V0^T) * exp(b_rho) f32     (rho bias folded multiplicatively)
    colsum+recip per-partition f; V^T = ev * recip -> TBs[:, :, 0:64] bf16
    attnT = eb * V^T                    -> TBs[:, :, 64:128] bf16
    PE-transpose TBs -> [V ; attnT] [n, f] psum bf16
    evac V -> vsb bf16; evac attnT -> attnp bf16 at permuted (hw,t) positions
    A-evac * Cm -> Urhs[:, :, 0:64]; [Cm|1] ext -> Urhs[:, :, 64:66]
    U psum [n, 66] += eb.T @ Urhs      (cols: U0^T | rowsumBC | rowsumB)
  Post: AB_T^T = (U0^T + rowsumBC x b_phi) / rowsumB;  Z = AB_T^T.T @ V (bf16 out)
        attn = attnp * (1/rowsumB) (bf16 out); host converts outputs to f32.
"""

import os
import sys

import numpy as np

for _p in ("/opt/trn_rl_repo", "/root/.axon_site/_ro/trn_rl_repo"):
    if _p not in sys.path and os.path.isdir(_p):
        sys.path.append(_p)

B, C, T, H, W = 8, 128, 16, 32, 32
M, N, CS = 64, 64, 64
THW = T * H * W  # 16384
NCORES = 8

FCHUNK = 128
NCHUNK = THW // FCHUNK  # 128
HWBLK = (H * W) // FCHUNK  # 8 chunks per t-plane
CB = 8  # chunks per super-chunk
NSC = NCHUNK // CB  # 32

_NC_CACHE = {}


def build_nc():
    import concourse.bacc as bacc
    import concourse.bass as bass
    import concourse.mybir as mybir
    import concourse.tile as tile
    from concourse.masks import make_identity

    fp32 = mybir.dt.float32
    bf16 = mybir.dt.bfloat16
    AF = mybir.ActivationFunctionType

    nc = bacc.Bacc(None, target_bir_lowering=False)

    x_d = nc.declare_dram_parameter("x", [C, THW], fp32, isOutput=False)
    wt_d = nc.declare_dram_parameter("wt_cat", [C, 3 * M], bf16, isOutput=False)
    w1t_d = nc.declare_dram_parameter("w1t", [CS, CS], fp32, isOutput=False)
    b1_d = nc.declare_dram_parameter("b1", [CS, 1], fp32, isOutput=False)
    w2t_d = nc.declare_dram_parameter("w2t", [CS, H * W], fp32, isOutput=False)
    b2pt_d = nc.declare_dram_parameter("b2pt", [FCHUNK, HWBLK], fp32, isOutput=False)
    cond_d = nc.declare_dram_parameter("cond", [CS, 1], fp32, isOutput=False)
    ebr_d = nc.declare_dram_parameter("ebrho_b", [FCHUNK, N], bf16, isOutput=False)
    bphi_d = nc.declare_dram_parameter("bphi_b", [N, M], fp32, isOutput=False)
    ebrc_d = nc.declare_dram_parameter("ebrho_col", [N, 1], fp32, isOutput=False)
    oz_d = nc.declare_dram_parameter("out_z", [M, THW], bf16, isOutput=True)
    oa_d = nc.declare_dram_parameter("out_attn", [N, THW], bf16, isOutput=True)

    with tile.TileContext(nc) as tc:
        import contextlib

        ctx = contextlib.ExitStack()
        with ctx:
            singles = ctx.enter_context(tc.tile_pool(name="singles", bufs=1))
            xpool = ctx.enter_context(tc.tile_pool(name="xp", bufs=3))
            work = ctx.enter_context(tc.tile_pool(name="work", bufs=3))
            psum_c = ctx.enter_context(tc.tile_pool(name="ps_c", bufs=1, space="PSUM"))
            psum_t = ctx.enter_context(tc.tile_pool(name="ps_t", bufs=2, space="PSUM"))
            psum_u = ctx.enter_context(tc.tile_pool(name="ps_u", bufs=1, space="PSUM"))
            psum_z = ctx.enter_context(tc.tile_pool(name="ps_z", bufs=1, space="PSUM"))

            ident = singles.tile([FCHUNK, FCHUNK], bf16)
            make_identity(nc, ident)

            wt_sb = singles.tile([C, 3 * M], bf16)
            nc.sync.dma_start(out=wt_sb, in_=wt_d[:, :])
            w1t_sb = singles.tile([CS, CS], fp32)
            nc.sync.dma_start(out=w1t_sb, in_=w1t_d[:, :])
            b1_sb = singles.tile([CS, 1], fp32)
            nc.sync.dma_start(out=b1_sb, in_=b1_d[:, :])
            w2t_sb = singles.tile([CS, H * W], fp32)
            nc.sync.dma_start(out=w2t_sb, in_=w2t_d[:, :])
            b2pt_sb = singles.tile([FCHUNK, HWBLK], fp32)
            nc.sync.dma_start(out=b2pt_sb, in_=b2pt_d[:, :])
            cond_sb = singles.tile([CS, 1], fp32)
            nc.sync.dma_start(out=cond_sb, in_=cond_d[:, :])
            ebr_sb = singles.tile([FCHUNK, 1, N], bf16)
            nc.sync.dma_start(out=ebr_sb, in_=ebr_d[:, :].rearrange("p (o n) -> p o n", o=1))
            bphi_sb = singles.tile([N, M], fp32)
            nc.sync.dma_start(out=bphi_sb, in_=bphi_d[:, :])
            ebrc_sb = singles.tile([N, 1], fp32)
            nc.sync.dma_start(out=ebrc_sb, in_=ebrc_d[:, :])

            ones_col = singles.tile([C, 1], fp32)
            nc.vector.memset(ones_col, 1.0)
            ones_row = singles.tile([1, C], fp32)
            nc.vector.memset(ones_row, 1.0)

            # ---------------- tiny FC-cond (fp32, negligible) ----------------
            ps_h1 = psum_t.tile([CS, 1], fp32, tag="pt")
            nc.tensor.matmul(ps_h1, w1t_sb, cond_sb)
            h1_sb = work.tile([CS, 1], fp32)
            nc.scalar.activation(h1_sb, ps_h1, AF.Relu, bias=b1_sb, scale=1.0)
            ps_h2 = psum_t.tile([FCHUNK, HWBLK], fp32, tag="pt")
            for k in range(HWBLK):
                nc.tensor.matmul(
                    ps_h2[:, k : k + 1],
                    w2t_sb[:, k * FCHUNK : (k + 1) * FCHUNK],
                    h1_sb,
                )
            h2_sb = work.tile([FCHUNK, HWBLK], fp32)
            nc.vector.tensor_add(h2_sb, ps_h2, b2pt_sb)
            nc.scalar.activation(h2_sb, h2_sb, AF.Relu)
            expc_sb = work.tile([FCHUNK, HWBLK], fp32)
            sc_sb = work.tile([FCHUNK, 1], fp32)
            nc.scalar.activation(expc_sb, h2_sb, AF.Exp, accum_out=sc_sb)
            ps_s = psum_t.tile([1, 1], fp32, tag="pt")
            nc.tensor.matmul(ps_s, ones_col, sc_sb)
            rc_sb = work.tile([1, 1], fp32)
            nc.vector.reciprocal(rc_sb, ps_s)
            ps_rc = psum_t.tile([FCHUNK, 1], fp32, tag="pt")
            nc.tensor.matmul(ps_rc, ones_row, rc_sb)
            rcp_sb = work.tile([FCHUNK, 1], fp32)
            nc.scalar.copy(rcp_sb, ps_rc)
            cm_sb = singles.tile([FCHUNK, HWBLK, 1], fp32)
            nc.vector.tensor_scalar_mul(
                cm_sb, in0=expc_sb.rearrange("p (k o) -> p k o", o=1), scalar1=rcp_sb
            )
            cmones_sb = singles.tile([FCHUNK, HWBLK, 2], bf16)
            nc.vector.memset(cmones_sb, 1.0)
            nc.vector.tensor_copy(cmones_sb[:, :, 0:1], cm_sb)

            # ---------------- x load + bf16 convert ----------------
            XPIECE = 2048
            NXD = THW // XPIECE  # 8
            xbf = singles.tile([C, THW], bf16)
            for i in range(NXD):
                s = i * XPIECE
                xt = xpool.tile([C, XPIECE], fp32, tag="xt")
                nc.sync.dma_start(out=xt, in_=x_d[:, s : s + XPIECE])
                if i % 2 == 0:
                    nc.scalar.copy(xbf[:, s : s + XPIECE], xt)
                else:
                    nc.vector.tensor_copy(xbf[:, s : s + XPIECE], xt)

            # ---------------- main loop ----------------
            vsb = singles.tile([N, THW], bf16)
            attq = singles.tile([N, THW], bf16)  # attn product, t-major
            ps_u_t = psum_u.tile([N, M + 2], fp32)

            for sc in range(NSC):
                c0 = sc * CB
                f0 = c0 * FCHUNK
                tidx = c0 // HWBLK
                hwb0 = c0 % HWBLK  # 0 or 4

                # per-chunk stride padded to 256 f32 so no slice crosses a PSUM bank
                pc = psum_c.tile([FCHUNK, CB, 4 * M], fp32, tag="pc")
                for j in range(CB):
                    nc.tensor.matmul(
                        pc[:, j, 0 : 3 * M],
                        xbf[:, f0 + j * FCHUNK : f0 + (j + 1) * FCHUNK],
                        wt_sb,
                    )

                ebv = work.tile([FCHUNK, CB, 2 * N], bf16, tag="ebv")
                nc.scalar.activation(ebv, pc[:, :, M : 3 * M], AF.Exp)
                eb = ebv[:, :, 0:N]
                ev = ebv[:, :, N : 2 * N]
                csum = work.tile([FCHUNK, CB], fp32, tag="csum")
                scr = work.tile([FCHUNK, CB, N], fp32, tag="scr")
                nc.vector.tensor_mul(scr, ev, ebr_sb.to_broadcast([FCHUNK, CB, N]))
                nc.vector.reduce_sum(csum, scr, axis=mybir.AxisListType.X)
                nc.vector.reciprocal(csum, csum)
                tbs = work.tile([FCHUNK, CB, 2 * N], bf16, tag="tbs")
                nc.vector.tensor_mul(
                    tbs[:, :, 0:N],
                    ev,
                    csum.rearrange("p (c o) -> p c o", o=1).to_broadcast(
                        [FCHUNK, CB, N]
                    ),
                )
                nc.vector.tensor_mul(tbs[:, :, N : 2 * N], eb, tbs[:, :, 0:N])

                pt = psum_t.tile([FCHUNK, CB, FCHUNK], bf16, tag="pt")
                for j in range(CB):
                    nc.tensor.transpose(pt[:, j, :], tbs[:, j, :], ident)

                nc.vector.tensor_copy(vsb[:, f0 : f0 + CB * FCHUNK], pt[0:N, :, :])
                nc.scalar.copy(attq[:, f0 : f0 + CB * FCHUNK], pt[N : 2 * N, :, :])

                urhs = work.tile([FCHUNK, CB, M + 2], bf16, tag="urhs")
                nc.vector.tensor_mul(
                    urhs[:, :, 0:M],
                    pc[:, :, 0:M],
                    cm_sb[:, hwb0 : hwb0 + CB, :].to_broadcast([FCHUNK, CB, M]),
                )
                nc.vector.tensor_copy(
                    urhs[:, :, M : M + 2], cmones_sb[:, hwb0 : hwb0 + CB, :]
                )
                for j in range(CB):
                    nc.tensor.matmul(
                        ps_u_t,
                        ebv[:, j, 0:N],
                        urhs[:, j, :],
                        start=(sc == 0 and j == 0),
                        stop=(sc == NSC - 1 and j == CB - 1),
                    )

            # ---------------- post loop ----------------
            u_sb = work.tile([N, M + 2], fp32)
            nc.scalar.copy(u_sb, ps_u_t)
            rb_sb = work.tile([N, 1], fp32)
            nc.vector.reciprocal(rb_sb, u_sb[:, M + 1 : M + 2])
            corr = work.tile([N, M], fp32)
            nc.vector.tensor_scalar_mul(corr, in0=bphi_sb, scalar1=u_sb[:, M : M + 1])
            nc.vector.tensor_add(corr, corr, u_sb[:, 0:M])
            rbw_sb = work.tile([N, 1], fp32)
            nc.vector.tensor_mul(rbw_sb, rb_sb, ebrc_sb)
            abt_sb = work.tile([N, M], bf16)
            nc.vector.tensor_scalar_mul(abt_sb, in0=corr, scalar1=rbw_sb)

            ZBLK = 512
            for bi in range(THW // ZBLK):
                s = bi * ZBLK
                pz = psum_z.tile([M, ZBLK], fp32, tag="pz")
                nc.tensor.matmul(pz, abt_sb, vsb[:, s : s + ZBLK])
                zt = work.tile([M, ZBLK], bf16, tag="zt")
                nc.scalar.copy(zt, pz)
                nc.sync.dma_start(out=oz_d[:, s : s + ZBLK], in_=zt)

            attq_v = attq.rearrange("p (t hw) -> p hw t", t=T)
            hhw = (H * W) // 2
            for ah in range(2):
                attnf = work.tile([N, THW // 2], bf16, tag="attnf")
                nc.vector.tensor_scalar_mul(
                    attnf.rearrange("p (hw t) -> p hw t", t=T),
                    in0=attq_v[:, ah * hhw : (ah + 1) * hhw, :],
                    scalar1=rbw_sb,
                )
                nc.sync.dma_start(
                    out=oa_d[:, ah * (THW // 2) : (ah + 1) * (THW // 2)], in_=attnf
                )


    nc.compile()
    return nc


def _prep_shared(W_phi, b_phi, W_theta, b_theta, W_rho, b_rho, W1, b1, W2, b2):
    import ml_dtypes

    f32 = np.float32
    bf = ml_dtypes.bfloat16
    wt_cat = np.concatenate(
        [np.asarray(W_phi).T, np.asarray(W_theta).T, np.asarray(W_rho).T], axis=1
    ).astype(f32)
    shared = {
        "wt_cat": np.ascontiguousarray(wt_cat.astype(bf)),
        "w1t": np.ascontiguousarray(np.asarray(W1).T.astype(f32)),
        "b1": np.asarray(b1).astype(f32).reshape(CS, 1),
        "w2t": np.ascontiguousarray(np.asarray(W2).T.astype(f32)),
        "b2pt": np.ascontiguousarray(
            np.asarray(b2).astype(f32).reshape(HWBLK, FCHUNK).T
        ),
        "ebrho_b": np.ascontiguousarray(
            np.tile(np.exp(np.asarray(b_rho).astype(f32))[None, :], (FCHUNK, 1)).astype(bf)
        ),
        "bphi_b": np.ascontiguousarray(
            np.tile(np.asarray(b_phi).astype(f32)[None, :], (N, 1))
        ),
        "ebrho_col": np.ascontiguousarray(
            np.exp(np.asarray(b_rho).astype(f32)).reshape(N, 1)
        ),
    }
    return shared


def _install_ntff_hook():
    """The container's antenv stub lacks axon_hooks; inject it and register
    the ctypes NTFF profiling hook so trace=True works under axon."""
    import sys
    import types

    if "antenv.axon_hooks" in sys.modules:
        return
    import antenv

    mod = types.ModuleType("antenv.axon_hooks")
    _state = {}
    mod.set_axon_ntff_profile_hook = lambda h: _state.__setitem__("h", h)
    mod.get_axon_ntff_profile_hook = lambda: _state.get("h")
    sys.modules["antenv.axon_hooks"] = mod
    antenv.axon_hooks = mod
    try:
        from trn_agent_boot.trn_boot import _ntff_profile_via_ctypes

        hook = _ntff_profile_via_ctypes("/opt/axon/libaxon_pjrt.so")
        if hook is not None:
            mod.set_axon_ntff_profile_hook(hook)
    except Exception as e:  # profiling degrades, run still works
        print("ntff hook install failed:", e)


def run_cores(inputs, trace=False):
    from concourse.bass_utils import run_bass_kernel_spmd

    if trace:
        _install_ntff_hook()

    if "nc" not in _NC_CACHE:
        _NC_CACHE["nc"] = build_nc()
    nc = _NC_CACHE["nc"]

    x_full = np.asarray(inputs["input"], dtype=np.float32).reshape(B, C, THW)
    cond_full = np.asarray(inputs["condition"], dtype=np.float32)
    shared = _prep_shared(
        inputs["W_phi"], inputs["b_phi"], inputs["W_theta"], inputs["b_theta"],
        inputs["W_rho"], inputs["b_rho"], inputs["W1"], inputs["b1"],
        inputs["W2"], inputs["b2"],
    )
    in_maps = []
    for b in range(NCORES):
        m = dict(shared)
        m["x"] = np.ascontiguousarray(x_full[b])
        m["cond"] = np.ascontiguousarray(cond_full[b].reshape(CS, 1))
        in_maps.append(m)

    kw = {}
    if trace:
        kw = dict(trace=True, trace_cores=[0])
    res = run_bass_kernel_spmd(nc, in_maps, core_ids=list(range(NCORES)), **kw)
    return res


def kernel(**inputs):
    res = run_cores(inputs, trace=False)
    z = np.stack(
        [np.asarray(r["out_z"], dtype=np.float32) for r in res.results], axis=0
    ).reshape(B, M, T, H, W)
    attn = np.stack(
        [np.asarray(r["out_attn"], dtype=np.float32) for r in res.results], axis=0
    ).reshape(B, N, H, W, T)
    return z, attn


# revision 26
# speedup vs baseline: 1.3813x; 1.2257x over previous
"""A2Net_Cond Trainium2 kernel: 8-core data-parallel over batch.

Per core (one batch element), bf16 compute / f32 accumulate:
  x [C=128, THW=16384] f32 -> bf16.
  Super-chunk loop (4 f-chunks of 128 per iteration):
    conv matmuls (x_c stationary bf16): psum [128f, 4, 192] = [A^T | B0^T | V0^T]
    eb = exp(B0^T) bf16 [f, n]          (theta bias dropped: softmax row-invariant)
    ev = exp(V0^T) * exp(b_rho) f32     (rho bias folded multiplicatively)
    colsum+recip per-partition f; V^T = ev * recip -> TBs[:, :, 0:64] bf16
    attnT = eb * V^T                    -> TBs[:, :, 64:128] bf16
    PE-transpose TBs -> [V ; attnT] [n, f] psum bf16
    evac V -> vsb bf16; evac attnT -> attnp bf16 at permuted (hw,t) positions
    A-evac * Cm -> Urhs[:, :, 0:64]; [Cm|1] ext -> Urhs[:, :, 64:66]
    U psum [n, 66] += eb.T @ Urhs      (cols: U0^T | rowsumBC | rowsumB)
  Post: AB_T^T = (U0^T + rowsumBC x b_phi) / rowsumB;  Z = AB_T^T.T @ V (bf16 out)
        attn = attnp * (1/rowsumB) (bf16 out); host converts outputs to f32.
"""

import os
import sys

import numpy as np

for _p in ("/opt/trn_rl_repo", "/root/.axon_site/_ro/trn_rl_repo"):
    if _p not in sys.path and os.path.isdir(_p):
        sys.path.append(_p)

B, C, T, H, W = 8, 128, 16, 32, 32
M, N, CS = 64, 64, 64
THW = T * H * W  # 16384
NCORES = 8

FCHUNK = 128
NCHUNK = THW // FCHUNK  # 128
HWBLK = (H * W) // FCHUNK  # 8 chunks per t-plane
CB = 8  # chunks per super-chunk
NSC = NCHUNK // CB  # 32

_NC_CACHE = {}


def build_nc():
    import concourse.bacc as bacc
    import concourse.bass as bass
    import concourse.mybir as mybir
    import concourse.tile as tile
    from concourse.masks import make_identity

    fp32 = mybir.dt.float32
    bf16 = mybir.dt.bfloat16
    AF = mybir.ActivationFunctionType

    nc = bacc.Bacc(None, target_bir_lowering=False)

    x_d = nc.declare_dram_parameter("x", [C, THW], fp32, isOutput=False)
    wt_d = nc.declare_dram_parameter("wt_cat", [C, 3 * M], bf16, isOutput=False)
    w1t_d = nc.declare_dram_parameter("w1t", [CS, CS], fp32, isOutput=False)
    b1_d = nc.declare_dram_parameter("b1", [CS, 1], fp32, isOutput=False)
    w2t_d = nc.declare_dram_parameter("w2t", [CS, H * W], fp32, isOutput=False)
    b2pt_d = nc.declare_dram_parameter("b2pt", [FCHUNK, HWBLK], fp32, isOutput=False)
    cond_d = nc.declare_dram_parameter("cond", [CS, 1], fp32, isOutput=False)
    ebr_d = nc.declare_dram_parameter("ebrho_b", [FCHUNK, N], bf16, isOutput=False)
    bphi_d = nc.declare_dram_parameter("bphi_b", [N, M], fp32, isOutput=False)
    ebrc_d = nc.declare_dram_parameter("ebrho_col", [N, 1], fp32, isOutput=False)
    oz_d = nc.declare_dram_parameter("out_z", [M, THW], bf16, isOutput=True)
    oa_d = nc.declare_dram_parameter("out_attn", [N, THW], bf16, isOutput=True)

    with tile.TileContext(nc) as tc:
        import contextlib

        ctx = contextlib.ExitStack()
        with ctx:
            singles = ctx.enter_context(tc.tile_pool(name="singles", bufs=1))
            xpool = ctx.enter_context(tc.tile_pool(name="xp", bufs=3))
            work = ctx.enter_context(tc.tile_pool(name="work", bufs=3))
            psum_a = ctx.enter_context(tc.tile_pool(name="ps_a", bufs=2, space="PSUM"))
            psum_bv = ctx.enter_context(tc.tile_pool(name="ps_bv", bufs=1, space="PSUM"))
            psum_t = ctx.enter_context(tc.tile_pool(name="ps_t", bufs=2, space="PSUM"))
            psum_u = ctx.enter_context(tc.tile_pool(name="ps_u", bufs=1, space="PSUM"))

            ident = singles.tile([FCHUNK, FCHUNK], bf16)
            make_identity(nc, ident)

            wt_sb = singles.tile([C, 3 * M], bf16)
            nc.sync.dma_start(out=wt_sb, in_=wt_d[:, :])
            w1t_sb = singles.tile([CS, CS], fp32)
            nc.sync.dma_start(out=w1t_sb, in_=w1t_d[:, :])
            b1_sb = singles.tile([CS, 1], fp32)
            nc.sync.dma_start(out=b1_sb, in_=b1_d[:, :])
            w2t_sb = singles.tile([CS, H * W], fp32)
            nc.sync.dma_start(out=w2t_sb, in_=w2t_d[:, :])
            b2pt_sb = singles.tile([FCHUNK, HWBLK], fp32)
            nc.sync.dma_start(out=b2pt_sb, in_=b2pt_d[:, :])
            cond_sb = singles.tile([CS, 1], fp32)
            nc.sync.dma_start(out=cond_sb, in_=cond_d[:, :])
            ebr_sb = singles.tile([FCHUNK, 1, N], bf16)
            nc.sync.dma_start(out=ebr_sb, in_=ebr_d[:, :].rearrange("p (o n) -> p o n", o=1))
            bphi_sb = singles.tile([N, M], fp32)
            nc.sync.dma_start(out=bphi_sb, in_=bphi_d[:, :])
            ebrc_sb = singles.tile([N, 1], fp32)
            nc.sync.dma_start(out=ebrc_sb, in_=ebrc_d[:, :])

            ones_col = singles.tile([C, 1], fp32)
            nc.vector.memset(ones_col, 1.0)
            ones_row = singles.tile([1, C], fp32)
            nc.vector.memset(ones_row, 1.0)

            # ---------------- tiny FC-cond (fp32, negligible) ----------------
            ps_h1 = psum_t.tile([CS, 1], fp32, tag="pt")
            nc.tensor.matmul(ps_h1, w1t_sb, cond_sb)
            h1_sb = work.tile([CS, 1], fp32)
            nc.scalar.activation(h1_sb, ps_h1, AF.Relu, bias=b1_sb, scale=1.0)
            ps_h2 = psum_t.tile([FCHUNK, HWBLK], fp32, tag="pt")
            for k in range(HWBLK):
                nc.tensor.matmul(
                    ps_h2[:, k : k + 1],
                    w2t_sb[:, k * FCHUNK : (k + 1) * FCHUNK],
                    h1_sb,
                )
            h2_sb = work.tile([FCHUNK, HWBLK], fp32)
            nc.vector.tensor_add(h2_sb, ps_h2, b2pt_sb)
            nc.scalar.activation(h2_sb, h2_sb, AF.Relu)
            expc_sb = work.tile([FCHUNK, HWBLK], fp32)
            sc_sb = work.tile([FCHUNK, 1], fp32)
            nc.scalar.activation(expc_sb, h2_sb, AF.Exp, accum_out=sc_sb)
            ps_s = psum_t.tile([1, 1], fp32, tag="pt")
            nc.tensor.matmul(ps_s, ones_col, sc_sb)
            rc_sb = work.tile([1, 1], fp32)
            nc.vector.reciprocal(rc_sb, ps_s)
            ps_rc = psum_t.tile([FCHUNK, 1], fp32, tag="pt")
            nc.tensor.matmul(ps_rc, ones_row, rc_sb)
            rcp_sb = work.tile([FCHUNK, 1], fp32)
            nc.scalar.copy(rcp_sb, ps_rc)
            cm_sb = singles.tile([FCHUNK, HWBLK, 1], fp32)
            nc.vector.tensor_scalar_mul(
                cm_sb, in0=expc_sb.rearrange("p (k o) -> p k o", o=1), scalar1=rcp_sb
            )
            cmones_sb = singles.tile([FCHUNK, HWBLK, 2], bf16)
            nc.vector.memset(cmones_sb, 1.0)
            nc.vector.tensor_copy(cmones_sb[:, :, 0:1], cm_sb)

            # ---------------- x load + bf16 convert ----------------
            XPIECE = 2048
            NXD = THW // XPIECE  # 8
            xbf = singles.tile([C, THW], bf16)
            for i in range(NXD):
                s = i * XPIECE
                xt = xpool.tile([C, XPIECE], fp32, tag="xt")
                nc.sync.dma_start(out=xt, in_=x_d[:, s : s + XPIECE])
                if i % 2 == 0:
                    nc.scalar.copy(xbf[:, s : s + XPIECE], xt)
                else:
                    nc.vector.tensor_copy(xbf[:, s : s + XPIECE], xt)

            # ---------------- main loop ----------------
            vsb = singles.tile([N, THW], bf16)
            attq = singles.tile([N, THW], bf16)  # attn product, t-major
            ps_u_t = psum_u.tile([N, M + 2], fp32)

            for sc in range(NSC):
                c0 = sc * CB
                f0 = c0 * FCHUNK
                tidx = c0 // HWBLK
                hwb0 = c0 % HWBLK  # 0 or 4

                pcA = psum_a.tile([FCHUNK, CB, M], fp32, tag="pcA")
                pcBV = psum_bv.tile([FCHUNK, CB, 2 * M], fp32, tag="pcBV")
                for j in range(CB):
                    xsl = xbf[:, f0 + j * FCHUNK : f0 + (j + 1) * FCHUNK]
                    nc.tensor.matmul(pcA[:, j, :], xsl, wt_sb[:, 0:M])
                    nc.tensor.matmul(pcBV[:, j, :], xsl, wt_sb[:, M : 3 * M])

                ebv = work.tile([FCHUNK, CB, 2 * N], bf16, tag="ebv")
                nc.scalar.activation(ebv, pcBV, AF.Exp)
                eb = ebv[:, :, 0:N]
                ev = ebv[:, :, N : 2 * N]
                csum = work.tile([FCHUNK, CB], fp32, tag="csum")
                scr = work.tile([FCHUNK, CB, N], fp32, tag="scr")
                nc.vector.tensor_mul(scr, ev, ebr_sb.to_broadcast([FCHUNK, CB, N]))
                nc.vector.reduce_sum(csum, scr, axis=mybir.AxisListType.X)
                nc.vector.reciprocal(csum, csum)
                tbs = work.tile([FCHUNK, CB, 2 * N], bf16, tag="tbs")
                nc.vector.tensor_mul(
                    tbs[:, :, 0:N],
                    ev,
                    csum.rearrange("p (c o) -> p c o", o=1).to_broadcast(
                        [FCHUNK, CB, N]
                    ),
                )
                nc.vector.tensor_mul(tbs[:, :, N : 2 * N], eb, tbs[:, :, 0:N])

                pt = psum_t.tile([FCHUNK, CB, FCHUNK], bf16, tag="pt")
                for j in range(CB):
                    nc.tensor.transpose(pt[:, j, :], tbs[:, j, :], ident)

                nc.vector.tensor_copy(vsb[:, f0 : f0 + CB * FCHUNK], pt[0:N, :, :])
                nc.scalar.copy(attq[:, f0 : f0 + CB * FCHUNK], pt[N : 2 * N, :, :])

                urhs = work.tile([FCHUNK, CB, M + 2], bf16, tag="urhs")
                nc.vector.tensor_mul(
                    urhs[:, :, 0:M],
                    pcA,
                    cm_sb[:, hwb0 : hwb0 + CB, :].to_broadcast([FCHUNK, CB, M]),
                )
                nc.vector.tensor_copy(
                    urhs[:, :, M : M + 2], cmones_sb[:, hwb0 : hwb0 + CB, :]
                )
                for j in range(CB):
                    nc.tensor.matmul(
                        ps_u_t,
                        ebv[:, j, 0:N],
                        urhs[:, j, :],
                        start=(sc == 0 and j == 0),
                        stop=(sc == NSC - 1 and j == CB - 1),
                    )

            # ---------------- post loop ----------------
            u_sb = work.tile([N, M + 2], fp32)
            nc.scalar.copy(u_sb, ps_u_t)
            rb_sb = work.tile([N, 1], fp32)
            nc.vector.reciprocal(rb_sb, u_sb[:, M + 1 : M + 2])
            corr = work.tile([N, M], fp32)
            nc.vector.tensor_scalar_mul(corr, in0=bphi_sb, scalar1=u_sb[:, M : M + 1])
            nc.vector.tensor_add(corr, corr, u_sb[:, 0:M])
            rbw_sb = work.tile([N, 1], fp32)
            nc.vector.tensor_mul(rbw_sb, rb_sb, ebrc_sb)
            abt_sb = work.tile([N, M], bf16)
            nc.vector.tensor_scalar_mul(abt_sb, in0=corr, scalar1=rbw_sb)

            ZBLK = 512
            for bi in range(THW // ZBLK):
                s = bi * ZBLK
                pz = psum_t.tile([M, ZBLK], fp32, tag="pt")
                nc.tensor.matmul(pz, abt_sb, vsb[:, s : s + ZBLK])
                zt = work.tile([M, ZBLK], bf16, tag="zt")
                nc.scalar.copy(zt, pz)
                nc.sync.dma_start(out=oz_d[:, s : s + ZBLK], in_=zt)

            attq_v = attq.rearrange("p (t hw) -> p hw t", t=T)
            hhw = (H * W) // 2
            for ah in range(2):
                attnf = work.tile([N, THW // 2], bf16, tag="attnf")
                nc.vector.tensor_scalar_mul(
                    attnf.rearrange("p (hw t) -> p hw t", t=T),
                    in0=attq_v[:, ah * hhw : (ah + 1) * hhw, :],
                    scalar1=rbw_sb,
                )
                nc.sync.dma_start(
                    out=oa_d[:, ah * (THW // 2) : (ah + 1) * (THW // 2)], in_=attnf
                )


    nc.compile()
    return nc


def _prep_shared(W_phi, b_phi, W_theta, b_theta, W_rho, b_rho, W1, b1, W2, b2):
    import ml_dtypes

    f32 = np.float32
    bf = ml_dtypes.bfloat16
    wt_cat = np.concatenate(
        [np.asarray(W_phi).T, np.asarray(W_theta).T, np.asarray(W_rho).T], axis=1
    ).astype(f32)
    shared = {
        "wt_cat": np.ascontiguousarray(wt_cat.astype(bf)),
        "w1t": np.ascontiguousarray(np.asarray(W1).T.astype(f32)),
        "b1": np.asarray(b1).astype(f32).reshape(CS, 1),
        "w2t": np.ascontiguousarray(np.asarray(W2).T.astype(f32)),
        "b2pt": np.ascontiguousarray(
            np.asarray(b2).astype(f32).reshape(HWBLK, FCHUNK).T
        ),
        "ebrho_b": np.ascontiguousarray(
            np.tile(np.exp(np.asarray(b_rho).astype(f32))[None, :], (FCHUNK, 1)).astype(bf)
        ),
        "bphi_b": np.ascontiguousarray(
            np.tile(np.asarray(b_phi).astype(f32)[None, :], (N, 1))
        ),
        "ebrho_col": np.ascontiguousarray(
            np.exp(np.asarray(b_rho).astype(f32)).reshape(N, 1)
        ),
    }
    return shared


def _install_ntff_hook():
    """The container's antenv stub lacks axon_hooks; inject it and register
    the ctypes NTFF profiling hook so trace=True works under axon."""
    import sys
    import types

    if "antenv.axon_hooks" in sys.modules:
        return
    import antenv

    mod = types.ModuleType("antenv.axon_hooks")
    _state = {}
    mod.set_axon_ntff_profile_hook = lambda h: _state.__setitem__("h", h)
    mod.get_axon_ntff_profile_hook = lambda: _state.get("h")
    sys.modules["antenv.axon_hooks"] = mod
    antenv.axon_hooks = mod
    try:
        from trn_agent_boot.trn_boot import _ntff_profile_via_ctypes

        hook = _ntff_profile_via_ctypes("/opt/axon/libaxon_pjrt.so")
        if hook is not None:
            mod.set_axon_ntff_profile_hook(hook)
    except Exception as e:  # profiling degrades, run still works
        print("ntff hook install failed:", e)


def run_cores(inputs, trace=False):
    from concourse.bass_utils import run_bass_kernel_spmd

    if trace:
        _install_ntff_hook()

    if "nc" not in _NC_CACHE:
        _NC_CACHE["nc"] = build_nc()
    nc = _NC_CACHE["nc"]

    x_full = np.asarray(inputs["input"], dtype=np.float32).reshape(B, C, THW)
    cond_full = np.asarray(inputs["condition"], dtype=np.float32)
    shared = _prep_shared(
        inputs["W_phi"], inputs["b_phi"], inputs["W_theta"], inputs["b_theta"],
        inputs["W_rho"], inputs["b_rho"], inputs["W1"], inputs["b1"],
        inputs["W2"], inputs["b2"],
    )
    in_maps = []
    for b in range(NCORES):
        m = dict(shared)
        m["x"] = np.ascontiguousarray(x_full[b])
        m["cond"] = np.ascontiguousarray(cond_full[b].reshape(CS, 1))
        in_maps.append(m)

    kw = {}
    if trace:
        kw = dict(trace=True, trace_cores=[0])
    res = run_bass_kernel_spmd(nc, in_maps, core_ids=list(range(NCORES)), **kw)
    return res


def kernel(**inputs):
    res = run_cores(inputs, trace=False)
    z = np.stack(
        [np.asarray(r["out_z"], dtype=np.float32) for r in res.results], axis=0
    ).reshape(B, M, T, H, W)
    attn = np.stack(
        [np.asarray(r["out_attn"], dtype=np.float32) for r in res.results], axis=0
    ).reshape(B, N, H, W, T)
    return z, attn


# revision 28
# speedup vs baseline: 1.3956x; 1.0103x over previous
"""A2Net_Cond Trainium2 kernel: 8-core data-parallel over batch.

Per core (one batch element), bf16 compute / f32 accumulate:
  x [C=128, THW=16384] f32 -> bf16.
  Super-chunk loop (4 f-chunks of 128 per iteration):
    conv matmuls (x_c stationary bf16): psum [128f, 4, 192] = [A^T | B0^T | V0^T]
    eb = exp(B0^T) bf16 [f, n]          (theta bias dropped: softmax row-invariant)
    ev = exp(V0^T) * exp(b_rho) f32     (rho bias folded multiplicatively)
    colsum+recip per-partition f; V^T = ev * recip -> TBs[:, :, 0:64] bf16
    attnT = eb * V^T                    -> TBs[:, :, 64:128] bf16
    PE-transpose TBs -> [V ; attnT] [n, f] psum bf16
    evac V -> vsb bf16; evac attnT -> attnp bf16 at permuted (hw,t) positions
    A-evac * Cm -> Urhs[:, :, 0:64]; [Cm|1] ext -> Urhs[:, :, 64:66]
    U psum [n, 66] += eb.T @ Urhs      (cols: U0^T | rowsumBC | rowsumB)
  Post: AB_T^T = (U0^T + rowsumBC x b_phi) / rowsumB;  Z = AB_T^T.T @ V (bf16 out)
        attn = attnp * (1/rowsumB) (bf16 out); host converts outputs to f32.
"""

import os
import sys

import numpy as np

for _p in ("/opt/trn_rl_repo", "/root/.axon_site/_ro/trn_rl_repo"):
    if _p not in sys.path and os.path.isdir(_p):
        sys.path.append(_p)

B, C, T, H, W = 8, 128, 16, 32, 32
M, N, CS = 64, 64, 64
THW = T * H * W  # 16384
NCORES = 8

FCHUNK = 128
NCHUNK = THW // FCHUNK  # 128
HWBLK = (H * W) // FCHUNK  # 8 chunks per t-plane
CB = 8  # chunks per super-chunk
NSC = NCHUNK // CB  # 32

_NC_CACHE = {}


def build_nc():
    import concourse.bacc as bacc
    import concourse.bass as bass
    import concourse.mybir as mybir
    import concourse.tile as tile
    from concourse.masks import make_identity

    fp32 = mybir.dt.float32
    bf16 = mybir.dt.bfloat16
    AF = mybir.ActivationFunctionType

    nc = bacc.Bacc(None, target_bir_lowering=False)

    x_d = nc.declare_dram_parameter("x", [C, THW], fp32, isOutput=False)
    wt_d = nc.declare_dram_parameter("wt_cat", [C, 3 * M], bf16, isOutput=False)
    w1t_d = nc.declare_dram_parameter("w1t", [CS, CS], fp32, isOutput=False)
    b1_d = nc.declare_dram_parameter("b1", [CS, 1], fp32, isOutput=False)
    w2t_d = nc.declare_dram_parameter("w2t", [CS, H * W], fp32, isOutput=False)
    b2pt_d = nc.declare_dram_parameter("b2pt", [FCHUNK, HWBLK], fp32, isOutput=False)
    cond_d = nc.declare_dram_parameter("cond", [CS, 1], fp32, isOutput=False)
    ebr_d = nc.declare_dram_parameter("ebrho_b", [FCHUNK, N], bf16, isOutput=False)
    bphi_d = nc.declare_dram_parameter("bphi_b", [N, M], fp32, isOutput=False)
    ebrc_d = nc.declare_dram_parameter("ebrho_col", [N, 1], fp32, isOutput=False)
    oz_d = nc.declare_dram_parameter("out_z", [M, THW], bf16, isOutput=True)
    oa_d = nc.declare_dram_parameter("out_attn", [N, THW], bf16, isOutput=True)
    orw_d = nc.declare_dram_parameter("out_rbw", [N, 1], fp32, isOutput=True)

    with tile.TileContext(nc) as tc:
        import contextlib

        ctx = contextlib.ExitStack()
        with ctx:
            singles = ctx.enter_context(tc.tile_pool(name="singles", bufs=1))
            xpool = ctx.enter_context(tc.tile_pool(name="xp", bufs=3))
            work = ctx.enter_context(tc.tile_pool(name="work", bufs=3))
            psum_a = ctx.enter_context(tc.tile_pool(name="ps_a", bufs=2, space="PSUM"))
            psum_bv = ctx.enter_context(tc.tile_pool(name="ps_bv", bufs=1, space="PSUM"))
            psum_t = ctx.enter_context(tc.tile_pool(name="ps_t", bufs=2, space="PSUM"))
            psum_u = ctx.enter_context(tc.tile_pool(name="ps_u", bufs=1, space="PSUM"))

            ident = singles.tile([FCHUNK, FCHUNK], bf16)
            make_identity(nc, ident)

            wt_sb = singles.tile([C, 3 * M], bf16)
            nc.sync.dma_start(out=wt_sb, in_=wt_d[:, :])
            w1t_sb = singles.tile([CS, CS], fp32)
            nc.sync.dma_start(out=w1t_sb, in_=w1t_d[:, :])
            b1_sb = singles.tile([CS, 1], fp32)
            nc.sync.dma_start(out=b1_sb, in_=b1_d[:, :])
            w2t_sb = singles.tile([CS, H * W], fp32)
            nc.sync.dma_start(out=w2t_sb, in_=w2t_d[:, :])
            b2pt_sb = singles.tile([FCHUNK, HWBLK], fp32)
            nc.sync.dma_start(out=b2pt_sb, in_=b2pt_d[:, :])
            cond_sb = singles.tile([CS, 1], fp32)
            nc.sync.dma_start(out=cond_sb, in_=cond_d[:, :])
            ebr_sb = singles.tile([FCHUNK, 1, N], bf16)
            nc.sync.dma_start(out=ebr_sb, in_=ebr_d[:, :].rearrange("p (o n) -> p o n", o=1))
            bphi_sb = singles.tile([N, M], fp32)
            nc.sync.dma_start(out=bphi_sb, in_=bphi_d[:, :])
            ebrc_sb = singles.tile([N, 1], fp32)
            nc.sync.dma_start(out=ebrc_sb, in_=ebrc_d[:, :])

            ones_col = singles.tile([C, 1], fp32)
            nc.vector.memset(ones_col, 1.0)
            ones_row = singles.tile([1, C], fp32)
            nc.vector.memset(ones_row, 1.0)

            # ---------------- tiny FC-cond (fp32, negligible) ----------------
            ps_h1 = psum_t.tile([CS, 1], fp32, tag="pt")
            nc.tensor.matmul(ps_h1, w1t_sb, cond_sb)
            h1_sb = work.tile([CS, 1], fp32)
            nc.scalar.activation(h1_sb, ps_h1, AF.Relu, bias=b1_sb, scale=1.0)
            ps_h2 = psum_t.tile([FCHUNK, HWBLK], fp32, tag="pt")
            for k in range(HWBLK):
                nc.tensor.matmul(
                    ps_h2[:, k : k + 1],
                    w2t_sb[:, k * FCHUNK : (k + 1) * FCHUNK],
                    h1_sb,
                )
            h2_sb = work.tile([FCHUNK, HWBLK], fp32)
            nc.vector.tensor_add(h2_sb, ps_h2, b2pt_sb)
            nc.scalar.activation(h2_sb, h2_sb, AF.Relu)
            expc_sb = work.tile([FCHUNK, HWBLK], fp32)
            sc_sb = work.tile([FCHUNK, 1], fp32)
            nc.scalar.activation(expc_sb, h2_sb, AF.Exp, accum_out=sc_sb)
            ps_s = psum_t.tile([1, 1], fp32, tag="pt")
            nc.tensor.matmul(ps_s, ones_col, sc_sb)
            rc_sb = work.tile([1, 1], fp32)
            nc.vector.reciprocal(rc_sb, ps_s)
            ps_rc = psum_t.tile([FCHUNK, 1], fp32, tag="pt")
            nc.tensor.matmul(ps_rc, ones_row, rc_sb)
            rcp_sb = work.tile([FCHUNK, 1], fp32)
            nc.scalar.copy(rcp_sb, ps_rc)
            cm_sb = singles.tile([FCHUNK, HWBLK, 1], fp32)
            nc.vector.tensor_scalar_mul(
                cm_sb, in0=expc_sb.rearrange("p (k o) -> p k o", o=1), scalar1=rcp_sb
            )
            cmones_sb = singles.tile([FCHUNK, HWBLK, 2], bf16)
            nc.vector.memset(cmones_sb, 1.0)
            nc.vector.tensor_copy(cmones_sb[:, :, 0:1], cm_sb)

            # ---------------- x load + bf16 convert ----------------
            XPIECE = 2048
            NXD = THW // XPIECE  # 8
            xbf = singles.tile([C, THW], bf16)
            for i in range(NXD):
                s = i * XPIECE
                xt = xpool.tile([C, XPIECE], fp32, tag="xt")
                nc.sync.dma_start(out=xt, in_=x_d[:, s : s + XPIECE])
                nc.scalar.copy(xbf[:, s : s + XPIECE], xt)

            # ---------------- main loop ----------------
            vsb = singles.tile([N, THW], bf16)
            attq = singles.tile([N, THW], bf16)  # attn product, t-major
            ps_u_t = psum_u.tile([N, M + 2], fp32)

            for sc in range(NSC):
                c0 = sc * CB
                f0 = c0 * FCHUNK
                tidx = c0 // HWBLK
                hwb0 = c0 % HWBLK  # 0 or 4

                pcA = psum_a.tile([FCHUNK, CB, M], fp32, tag="pcA")
                pcBV = psum_bv.tile([FCHUNK, CB, 2 * M], fp32, tag="pcBV")
                for j in range(CB):
                    xsl = xbf[:, f0 + j * FCHUNK : f0 + (j + 1) * FCHUNK]
                    nc.tensor.matmul(pcA[:, j, :], xsl, wt_sb[:, 0:M])
                    nc.tensor.matmul(pcBV[:, j, :], xsl, wt_sb[:, M : 3 * M])

                ebv = work.tile([FCHUNK, CB, 2 * N], bf16, tag="ebv")
                nc.scalar.activation(ebv, pcBV, AF.Exp)
                eb = ebv[:, :, 0:N]
                ev = ebv[:, :, N : 2 * N]
                csum = work.tile([FCHUNK, CB], fp32, tag="csum")
                scr = work.tile([FCHUNK, CB, N], fp32, tag="scr")
                nc.vector.tensor_mul(scr, ev, ebr_sb.to_broadcast([FCHUNK, CB, N]))
                nc.vector.reduce_sum(csum, scr, axis=mybir.AxisListType.X)
                nc.vector.reciprocal(csum, csum)
                tbs = work.tile([FCHUNK, CB, 2 * N], bf16, tag="tbs")
                nc.vector.tensor_mul(
                    tbs[:, :, 0:N],
                    ev,
                    csum.rearrange("p (c o) -> p c o", o=1).to_broadcast(
                        [FCHUNK, CB, N]
                    ),
                )
                nc.vector.tensor_mul(tbs[:, :, N : 2 * N], eb, tbs[:, :, 0:N])

                pt = psum_t.tile([FCHUNK, CB, FCHUNK], bf16, tag="pt")
                for j in range(CB):
                    nc.tensor.transpose(pt[:, j, :], tbs[:, j, :], ident)

                nc.vector.tensor_copy(vsb[:, f0 : f0 + CB * FCHUNK], pt[0:N, :, :])
                nc.scalar.copy(attq[:, f0 : f0 + CB * FCHUNK], pt[N : 2 * N, :, :])

                urhs = work.tile([FCHUNK, CB, M + 2], bf16, tag="urhs")
                nc.vector.tensor_mul(
                    urhs[:, :, 0:M],
                    pcA,
                    cm_sb[:, hwb0 : hwb0 + CB, :].to_broadcast([FCHUNK, CB, M]),
                )
                nc.vector.tensor_copy(
                    urhs[:, :, M : M + 2], cmones_sb[:, hwb0 : hwb0 + CB, :]
                )
                for j in range(CB):
                    nc.tensor.matmul(
                        ps_u_t,
                        ebv[:, j, 0:N],
                        urhs[:, j, :],
                        start=(sc == 0 and j == 0),
                        stop=(sc == NSC - 1 and j == CB - 1),
                    )

            # ---------------- post loop ----------------
            u_sb = work.tile([N, M + 2], fp32)
            nc.scalar.copy(u_sb, ps_u_t)
            rb_sb = work.tile([N, 1], fp32)
            nc.vector.reciprocal(rb_sb, u_sb[:, M + 1 : M + 2])
            corr = work.tile([N, M], fp32)
            nc.vector.tensor_scalar_mul(corr, in0=bphi_sb, scalar1=u_sb[:, M : M + 1])
            nc.vector.tensor_add(corr, corr, u_sb[:, 0:M])
            rbw_sb = work.tile([N, 1], fp32)
            nc.vector.tensor_mul(rbw_sb, rb_sb, ebrc_sb)
            abt_sb = work.tile([N, M], bf16)
            nc.vector.tensor_scalar_mul(abt_sb, in0=corr, scalar1=rbw_sb)

            ZBLK = 512
            for bi in range(THW // ZBLK):
                s = bi * ZBLK
                pz = psum_t.tile([M, ZBLK], fp32, tag="pt")
                nc.tensor.matmul(pz, abt_sb, vsb[:, s : s + ZBLK])
                zt = work.tile([M, ZBLK], bf16, tag="zt")
                nc.scalar.copy(zt, pz)
                nc.sync.dma_start(out=oz_d[:, s : s + ZBLK], in_=zt)

            nc.sync.dma_start(out=orw_d[:, :], in_=rbw_sb)
            for ah in range(2):
                s = ah * (THW // 2)
                e = s + THW // 2
                nc.sync.dma_start(out=oa_d[:, s:e], in_=attq[:, s:e])


    nc.compile()
    return nc


def _prep_shared(W_phi, b_phi, W_theta, b_theta, W_rho, b_rho, W1, b1, W2, b2):
    import ml_dtypes

    f32 = np.float32
    bf = ml_dtypes.bfloat16
    wt_cat = np.concatenate(
        [np.asarray(W_phi).T, np.asarray(W_theta).T, np.asarray(W_rho).T], axis=1
    ).astype(f32)
    shared = {
        "wt_cat": np.ascontiguousarray(wt_cat.astype(bf)),
        "w1t": np.ascontiguousarray(np.asarray(W1).T.astype(f32)),
        "b1": np.asarray(b1).astype(f32).reshape(CS, 1),
        "w2t": np.ascontiguousarray(np.asarray(W2).T.astype(f32)),
        "b2pt": np.ascontiguousarray(
            np.asarray(b2).astype(f32).reshape(HWBLK, FCHUNK).T
        ),
        "ebrho_b": np.ascontiguousarray(
            np.tile(np.exp(np.asarray(b_rho).astype(f32))[None, :], (FCHUNK, 1)).astype(bf)
        ),
        "bphi_b": np.ascontiguousarray(
            np.tile(np.asarray(b_phi).astype(f32)[None, :], (N, 1))
        ),
        "ebrho_col": np.ascontiguousarray(
            np.exp(np.asarray(b_rho).astype(f32)).reshape(N, 1)
        ),
    }
    return shared


def _install_ntff_hook():
    """The container's antenv stub lacks axon_hooks; inject it and register
    the ctypes NTFF profiling hook so trace=True works under axon."""
    import sys
    import types

    if "antenv.axon_hooks" in sys.modules:
        return
    import antenv

    mod = types.ModuleType("antenv.axon_hooks")
    _state = {}
    mod.set_axon_ntff_profile_hook = lambda h: _state.__setitem__("h", h)
    mod.get_axon_ntff_profile_hook = lambda: _state.get("h")
    sys.modules["antenv.axon_hooks"] = mod
    antenv.axon_hooks = mod
    try:
        from trn_agent_boot.trn_boot import _ntff_profile_via_ctypes

        hook = _ntff_profile_via_ctypes("/opt/axon/libaxon_pjrt.so")
        if hook is not None:
            mod.set_axon_ntff_profile_hook(hook)
    except Exception as e:  # profiling degrades, run still works
        print("ntff hook install failed:", e)


def run_cores(inputs, trace=False):
    from concourse.bass_utils import run_bass_kernel_spmd

    if trace:
        _install_ntff_hook()

    if "nc" not in _NC_CACHE:
        _NC_CACHE["nc"] = build_nc()
    nc = _NC_CACHE["nc"]

    x_full = np.asarray(inputs["input"], dtype=np.float32).reshape(B, C, THW)
    cond_full = np.asarray(inputs["condition"], dtype=np.float32)
    shared = _prep_shared(
        inputs["W_phi"], inputs["b_phi"], inputs["W_theta"], inputs["b_theta"],
        inputs["W_rho"], inputs["b_rho"], inputs["W1"], inputs["b1"],
        inputs["W2"], inputs["b2"],
    )
    in_maps = []
    for b in range(NCORES):
        m = dict(shared)
        m["x"] = np.ascontiguousarray(x_full[b])
        m["cond"] = np.ascontiguousarray(cond_full[b].reshape(CS, 1))
        in_maps.append(m)

    kw = {}
    if trace:
        kw = dict(trace=True, trace_cores=[0])
    res = run_bass_kernel_spmd(nc, in_maps, core_ids=list(range(NCORES)), **kw)
    return res


def kernel(**inputs):
    res = run_cores(inputs, trace=False)
    z = np.stack(
        [np.asarray(r["out_z"], dtype=np.float32) for r in res.results], axis=0
    ).reshape(B, M, T, H, W)
    # out_attn is raw eb*V in [n, (t, hw)] order; scale by rbw and permute on host
    attn = np.empty((B, N, H, W, T), dtype=np.float32)
    for b in range(B):
        a = np.asarray(res.results[b]["out_attn"], dtype=np.float32)
        rw = np.asarray(res.results[b]["out_rbw"], dtype=np.float32)
        a = (a * rw).reshape(N, T, H, W)
        attn[b] = a.transpose(0, 2, 3, 1)
    return z, attn


# revision 29
# speedup vs baseline: 1.4826x; 1.0623x over previous
"""A2Net_Cond Trainium2 kernel: 8-core data-parallel over batch.

Per core (one batch element), bf16 compute / f32 accumulate:
  x [C=128, THW=16384] f32 -> bf16.
  Super-chunk loop (4 f-chunks of 128 per iteration):
    conv matmuls (x_c stationary bf16): psum [128f, 4, 192] = [A^T | B0^T | V0^T]
    eb = exp(B0^T) bf16 [f, n]          (theta bias dropped: softmax row-invariant)
    ev = exp(V0^T) * exp(b_rho) f32     (rho bias folded multiplicatively)
    colsum+recip per-partition f; V^T = ev * recip -> TBs[:, :, 0:64] bf16
    attnT = eb * V^T                    -> TBs[:, :, 64:128] bf16
    PE-transpose TBs -> [V ; attnT] [n, f] psum bf16
    evac V -> vsb bf16; evac attnT -> attnp bf16 at permuted (hw,t) positions
    A-evac * Cm -> Urhs[:, :, 0:64]; [Cm|1] ext -> Urhs[:, :, 64:66]
    U psum [n, 66] += eb.T @ Urhs      (cols: U0^T | rowsumBC | rowsumB)
  Post: AB_T^T = (U0^T + rowsumBC x b_phi) / rowsumB;  Z = AB_T^T.T @ V (bf16 out)
        attn = attnp * (1/rowsumB) (bf16 out); host converts outputs to f32.
"""

import os
import sys

import numpy as np

for _p in ("/opt/trn_rl_repo", "/root/.axon_site/_ro/trn_rl_repo"):
    if _p not in sys.path and os.path.isdir(_p):
        sys.path.append(_p)

B, C, T, H, W = 8, 128, 16, 32, 32
M, N, CS = 64, 64, 64
THW = T * H * W  # 16384
NCORES = 8

FCHUNK = 128
NCHUNK = THW // FCHUNK  # 128
HWBLK = (H * W) // FCHUNK  # 8 chunks per t-plane
CB = 8  # chunks per super-chunk
NSC = NCHUNK // CB  # 32

_NC_CACHE = {}


def build_nc():
    import concourse.bacc as bacc
    import concourse.bass as bass
    import concourse.mybir as mybir
    import concourse.tile as tile
    from concourse.masks import make_identity

    fp32 = mybir.dt.float32
    bf16 = mybir.dt.bfloat16
    AF = mybir.ActivationFunctionType

    nc = bacc.Bacc(None, target_bir_lowering=False)

    x_d = nc.declare_dram_parameter("x", [C, THW], fp32, isOutput=False)
    wt_d = nc.declare_dram_parameter("wt_cat", [C, 3 * M], bf16, isOutput=False)
    w1t_d = nc.declare_dram_parameter("w1t", [CS, CS], fp32, isOutput=False)
    b1_d = nc.declare_dram_parameter("b1", [CS, 1], fp32, isOutput=False)
    w2t_d = nc.declare_dram_parameter("w2t", [CS, H * W], fp32, isOutput=False)
    b2pt_d = nc.declare_dram_parameter("b2pt", [FCHUNK, HWBLK], fp32, isOutput=False)
    cond_d = nc.declare_dram_parameter("cond", [CS, 1], fp32, isOutput=False)
    ebr_d = nc.declare_dram_parameter("ebrho_b", [FCHUNK, N], bf16, isOutput=False)
    bphi_d = nc.declare_dram_parameter("bphi_b", [N, M], fp32, isOutput=False)
    ebrc_d = nc.declare_dram_parameter("ebrho_col", [N, 1], fp32, isOutput=False)
    oz_d = nc.declare_dram_parameter("out_z", [M, THW], bf16, isOutput=True)
    oa_d = nc.declare_dram_parameter("out_attn", [N, THW], bf16, isOutput=True)
    orw_d = nc.declare_dram_parameter("out_rbw", [N, 1], fp32, isOutput=True)

    with tile.TileContext(nc) as tc:
        import contextlib

        ctx = contextlib.ExitStack()
        with ctx:
            singles = ctx.enter_context(tc.tile_pool(name="singles", bufs=1))
            xpool = ctx.enter_context(tc.tile_pool(name="xp", bufs=3))
            work = ctx.enter_context(tc.tile_pool(name="work", bufs=4))
            psum_a = ctx.enter_context(tc.tile_pool(name="ps_a", bufs=2, space="PSUM"))
            psum_bv = ctx.enter_context(tc.tile_pool(name="ps_bv", bufs=1, space="PSUM"))
            psum_t = ctx.enter_context(tc.tile_pool(name="ps_t", bufs=3, space="PSUM"))
            psum_u = ctx.enter_context(tc.tile_pool(name="ps_u", bufs=1, space="PSUM"))

            ident = singles.tile([FCHUNK, FCHUNK], bf16)
            make_identity(nc, ident)

            wt_sb = singles.tile([C, 3 * M], bf16)
            nc.sync.dma_start(out=wt_sb, in_=wt_d[:, :])
            w1t_sb = singles.tile([CS, CS], fp32)
            nc.sync.dma_start(out=w1t_sb, in_=w1t_d[:, :])
            b1_sb = singles.tile([CS, 1], fp32)
            nc.sync.dma_start(out=b1_sb, in_=b1_d[:, :])
            w2t_sb = singles.tile([CS, H * W], fp32)
            nc.sync.dma_start(out=w2t_sb, in_=w2t_d[:, :])
            b2pt_sb = singles.tile([FCHUNK, HWBLK], fp32)
            nc.sync.dma_start(out=b2pt_sb, in_=b2pt_d[:, :])
            cond_sb = singles.tile([CS, 1], fp32)
            nc.sync.dma_start(out=cond_sb, in_=cond_d[:, :])
            ebr_sb = singles.tile([FCHUNK, 1, N], bf16)
            nc.sync.dma_start(out=ebr_sb, in_=ebr_d[:, :].rearrange("p (o n) -> p o n", o=1))
            bphi_sb = singles.tile([N, M], fp32)
            nc.sync.dma_start(out=bphi_sb, in_=bphi_d[:, :])
            ebrc_sb = singles.tile([N, 1], fp32)
            nc.sync.dma_start(out=ebrc_sb, in_=ebrc_d[:, :])

            ones_col = singles.tile([C, 1], fp32)
            nc.vector.memset(ones_col, 1.0)
            ones_row = singles.tile([1, C], fp32)
            nc.vector.memset(ones_row, 1.0)

            # ---------------- tiny FC-cond (fp32, negligible) ----------------
            ps_h1 = psum_t.tile([CS, 1], fp32, tag="pt")
            nc.tensor.matmul(ps_h1, w1t_sb, cond_sb)
            h1_sb = work.tile([CS, 1], fp32)
            nc.scalar.activation(h1_sb, ps_h1, AF.Relu, bias=b1_sb, scale=1.0)
            ps_h2 = psum_t.tile([FCHUNK, HWBLK], fp32, tag="pt")
            for k in range(HWBLK):
                nc.tensor.matmul(
                    ps_h2[:, k : k + 1],
                    w2t_sb[:, k * FCHUNK : (k + 1) * FCHUNK],
                    h1_sb,
                )
            h2_sb = work.tile([FCHUNK, HWBLK], fp32)
            nc.vector.tensor_add(h2_sb, ps_h2, b2pt_sb)
            nc.scalar.activation(h2_sb, h2_sb, AF.Relu)
            expc_sb = work.tile([FCHUNK, HWBLK], fp32)
            sc_sb = work.tile([FCHUNK, 1], fp32)
            nc.scalar.activation(expc_sb, h2_sb, AF.Exp, accum_out=sc_sb)
            ps_s = psum_t.tile([1, 1], fp32, tag="pt")
            nc.tensor.matmul(ps_s, ones_col, sc_sb)
            rc_sb = work.tile([1, 1], fp32)
            nc.vector.reciprocal(rc_sb, ps_s)
            ps_rc = psum_t.tile([FCHUNK, 1], fp32, tag="pt")
            nc.tensor.matmul(ps_rc, ones_row, rc_sb)
            rcp_sb = work.tile([FCHUNK, 1], fp32)
            nc.scalar.copy(rcp_sb, ps_rc)
            cm_sb = singles.tile([FCHUNK, HWBLK, 1], fp32)
            nc.vector.tensor_scalar_mul(
                cm_sb, in0=expc_sb.rearrange("p (k o) -> p k o", o=1), scalar1=rcp_sb
            )
            cmones_sb = singles.tile([FCHUNK, HWBLK, 2], bf16)
            nc.vector.memset(cmones_sb, 1.0)
            nc.vector.tensor_copy(cmones_sb[:, :, 0:1], cm_sb)

            # ---------------- x load + bf16 convert ----------------
            XPIECE = 2048
            NXD = THW // XPIECE  # 8
            xbf = singles.tile([C, THW], bf16)
            for i in range(NXD):
                s = i * XPIECE
                xt = xpool.tile([C, XPIECE], fp32, tag="xt")
                nc.sync.dma_start(out=xt, in_=x_d[:, s : s + XPIECE])
                nc.vector.tensor_copy(xbf[:, s : s + XPIECE], xt)

            # ---------------- main loop ----------------
            vq = singles.tile([2 * N, THW], bf16)  # rows 0:64 V, 64:128 attn raw
            ps_u_t = psum_u.tile([N, M + 2], fp32)

            for sc in range(NSC):
                c0 = sc * CB
                f0 = c0 * FCHUNK
                tidx = c0 // HWBLK
                hwb0 = c0 % HWBLK  # 0 or 4

                pcA = psum_a.tile([FCHUNK, CB, M], fp32, tag="pcA")
                pcBV = psum_bv.tile([FCHUNK, CB, 2 * M], fp32, tag="pcBV")
                for j in range(CB):
                    xsl = xbf[:, f0 + j * FCHUNK : f0 + (j + 1) * FCHUNK]
                    nc.tensor.matmul(pcA[:, j, :], xsl, wt_sb[:, 0:M])
                    nc.tensor.matmul(pcBV[:, j, :], xsl, wt_sb[:, M : 3 * M])

                ebv = work.tile([FCHUNK, CB, 2 * N], bf16, tag="ebv")
                nc.scalar.activation(ebv, pcBV, AF.Exp)
                eb = ebv[:, :, 0:N]
                ev = ebv[:, :, N : 2 * N]
                csum = work.tile([FCHUNK, CB], fp32, tag="csum")
                scr = work.tile([FCHUNK, CB, N], fp32, tag="scr")
                nc.vector.tensor_mul(scr, ev, ebr_sb.to_broadcast([FCHUNK, CB, N]))
                nc.vector.reduce_sum(csum, scr, axis=mybir.AxisListType.X)
                nc.vector.reciprocal(csum, csum)
                tbs = work.tile([FCHUNK, CB, 2 * N], bf16, tag="tbs")
                nc.vector.tensor_mul(
                    tbs[:, :, 0:N],
                    ev,
                    csum.rearrange("p (c o) -> p c o", o=1).to_broadcast(
                        [FCHUNK, CB, N]
                    ),
                )
                nc.vector.tensor_mul(tbs[:, :, N : 2 * N], eb, tbs[:, :, 0:N])

                pt = psum_t.tile([FCHUNK, CB, FCHUNK], bf16, tag="pt")
                for j in range(CB):
                    nc.tensor.transpose(pt[:, j, :], tbs[:, j, :], ident)

                nc.scalar.copy(vq[:, f0 : f0 + CB * FCHUNK], pt)

                urhs = work.tile([FCHUNK, CB, M + 2], bf16, tag="urhs")
                nc.vector.tensor_mul(
                    urhs[:, :, 0:M],
                    pcA,
                    cm_sb[:, hwb0 : hwb0 + CB, :].to_broadcast([FCHUNK, CB, M]),
                )
                nc.vector.tensor_copy(
                    urhs[:, :, M : M + 2], cmones_sb[:, hwb0 : hwb0 + CB, :]
                )
                for j in range(CB):
                    nc.tensor.matmul(
                        ps_u_t,
                        ebv[:, j, 0:N],
                        urhs[:, j, :],
                        start=(sc == 0 and j == 0),
                        stop=(sc == NSC - 1 and j == CB - 1),
                    )

            # ---------------- post loop ----------------
            u_sb = work.tile([N, M + 2], fp32)
            nc.scalar.copy(u_sb, ps_u_t)
            rb_sb = work.tile([N, 1], fp32)
            nc.vector.reciprocal(rb_sb, u_sb[:, M + 1 : M + 2])
            corr = work.tile([N, M], fp32)
            nc.vector.tensor_scalar_mul(corr, in0=bphi_sb, scalar1=u_sb[:, M : M + 1])
            nc.vector.tensor_add(corr, corr, u_sb[:, 0:M])
            rbw_sb = work.tile([N, 1], fp32)
            nc.vector.tensor_mul(rbw_sb, rb_sb, ebrc_sb)
            abt_sb = work.tile([N, M], bf16)
            nc.vector.tensor_scalar_mul(abt_sb, in0=corr, scalar1=rbw_sb)

            ZBLK = 512
            for bi in range(THW // ZBLK):
                s = bi * ZBLK
                pz = psum_t.tile([M, ZBLK], fp32, tag="pt")
                nc.tensor.matmul(pz, abt_sb, vq[0:N, s : s + ZBLK])
                zt = work.tile([M, ZBLK], bf16, tag="zt")
                if bi % 4 == 3:
                    nc.vector.tensor_copy(zt, pz)
                else:
                    nc.scalar.copy(zt, pz)
                nc.sync.dma_start(out=oz_d[:, s : s + ZBLK], in_=zt)

            nc.sync.dma_start(out=orw_d[:, :], in_=rbw_sb)
            for ah in range(2):
                s = ah * (THW // 2)
                e = s + THW // 2
                nc.sync.dma_start(out=oa_d[:, s:e], in_=vq[N : 2 * N, s:e])


    nc.compile()
    return nc


def _prep_shared(W_phi, b_phi, W_theta, b_theta, W_rho, b_rho, W1, b1, W2, b2):
    import ml_dtypes

    f32 = np.float32
    bf = ml_dtypes.bfloat16
    wt_cat = np.concatenate(
        [np.asarray(W_phi).T, np.asarray(W_theta).T, np.asarray(W_rho).T], axis=1
    ).astype(f32)
    shared = {
        "wt_cat": np.ascontiguousarray(wt_cat.astype(bf)),
        "w1t": np.ascontiguousarray(np.asarray(W1).T.astype(f32)),
        "b1": np.asarray(b1).astype(f32).reshape(CS, 1),
        "w2t": np.ascontiguousarray(np.asarray(W2).T.astype(f32)),
        "b2pt": np.ascontiguousarray(
            np.asarray(b2).astype(f32).reshape(HWBLK, FCHUNK).T
        ),
        "ebrho_b": np.ascontiguousarray(
            np.tile(np.exp(np.asarray(b_rho).astype(f32))[None, :], (FCHUNK, 1)).astype(bf)
        ),
        "bphi_b": np.ascontiguousarray(
            np.tile(np.asarray(b_phi).astype(f32)[None, :], (N, 1))
        ),
        "ebrho_col": np.ascontiguousarray(
            np.exp(np.asarray(b_rho).astype(f32)).reshape(N, 1)
        ),
    }
    return shared


def _install_ntff_hook():
    """The container's antenv stub lacks axon_hooks; inject it and register
    the ctypes NTFF profiling hook so trace=True works under axon."""
    import sys
    import types

    if "antenv.axon_hooks" in sys.modules:
        return
    import antenv

    mod = types.ModuleType("antenv.axon_hooks")
    _state = {}
    mod.set_axon_ntff_profile_hook = lambda h: _state.__setitem__("h", h)
    mod.get_axon_ntff_profile_hook = lambda: _state.get("h")
    sys.modules["antenv.axon_hooks"] = mod
    antenv.axon_hooks = mod
    try:
        from trn_agent_boot.trn_boot import _ntff_profile_via_ctypes

        hook = _ntff_profile_via_ctypes("/opt/axon/libaxon_pjrt.so")
        if hook is not None:
            mod.set_axon_ntff_profile_hook(hook)
    except Exception as e:  # profiling degrades, run still works
        print("ntff hook install failed:", e)


def run_cores(inputs, trace=False):
    from concourse.bass_utils import run_bass_kernel_spmd

    if trace:
        _install_ntff_hook()

    if "nc" not in _NC_CACHE:
        _NC_CACHE["nc"] = build_nc()
    nc = _NC_CACHE["nc"]

    x_full = np.asarray(inputs["input"], dtype=np.float32).reshape(B, C, THW)
    cond_full = np.asarray(inputs["condition"], dtype=np.float32)
    shared = _prep_shared(
        inputs["W_phi"], inputs["b_phi"], inputs["W_theta"], inputs["b_theta"],
        inputs["W_rho"], inputs["b_rho"], inputs["W1"], inputs["b1"],
        inputs["W2"], inputs["b2"],
    )
    in_maps = []
    for b in range(NCORES):
        m = dict(shared)
        m["x"] = np.ascontiguousarray(x_full[b])
        m["cond"] = np.ascontiguousarray(cond_full[b].reshape(CS, 1))
        in_maps.append(m)

    kw = {}
    if trace:
        kw = dict(trace=True, trace_cores=[0])
    res = run_bass_kernel_spmd(nc, in_maps, core_ids=list(range(NCORES)), **kw)
    return res


def kernel(**inputs):
    res = run_cores(inputs, trace=False)
    z = np.stack(
        [np.asarray(r["out_z"], dtype=np.float32) for r in res.results], axis=0
    ).reshape(B, M, T, H, W)
    # out_attn is raw eb*V in [n, (t, hw)] order; scale by rbw and permute on host
    attn = np.empty((B, N, H, W, T), dtype=np.float32)
    for b in range(B):
        a = np.asarray(res.results[b]["out_attn"], dtype=np.float32)
        rw = np.asarray(res.results[b]["out_rbw"], dtype=np.float32)
        a = (a * rw).reshape(N, T, H, W)
        attn[b] = a.transpose(0, 2, 3, 1)
    return z, attn


# revision 30
# speedup vs baseline: 1.5794x; 1.0653x over previous
"""A2Net_Cond Trainium2 kernel: 8-core data-parallel over batch.

Per core (one batch element), bf16 compute / f32 accumulate:
  x [C=128, THW=16384] f32 -> bf16.
  Super-chunk loop (4 f-chunks of 128 per iteration):
    conv matmuls (x_c stationary bf16): psum [128f, 4, 192] = [A^T | B0^T | V0^T]
    eb = exp(B0^T) bf16 [f, n]          (theta bias dropped: softmax row-invariant)
    ev = exp(V0^T) * exp(b_rho) f32     (rho bias folded multiplicatively)
    colsum+recip per-partition f; V^T = ev * recip -> TBs[:, :, 0:64] bf16
    attnT = eb * V^T                    -> TBs[:, :, 64:128] bf16
    PE-transpose TBs -> [V ; attnT] [n, f] psum bf16
    evac V -> vsb bf16; evac attnT -> attnp bf16 at permuted (hw,t) positions
    A-evac * Cm -> Urhs[:, :, 0:64]; [Cm|1] ext -> Urhs[:, :, 64:66]
    U psum [n, 66] += eb.T @ Urhs      (cols: U0^T | rowsumBC | rowsumB)
  Post: AB_T^T = (U0^T + rowsumBC x b_phi) / rowsumB;  Z = AB_T^T.T @ V (bf16 out)
        attn = attnp * (1/rowsumB) (bf16 out); host converts outputs to f32.
"""

import os
import sys

import numpy as np

for _p in ("/opt/trn_rl_repo", "/root/.axon_site/_ro/trn_rl_repo"):
    if _p not in sys.path and os.path.isdir(_p):
        sys.path.append(_p)

B, C, T, H, W = 8, 128, 16, 32, 32
M, N, CS = 64, 64, 64
THW = T * H * W  # 16384
NCORES = 8

FCHUNK = 128
NCHUNK = THW // FCHUNK  # 128
HWBLK = (H * W) // FCHUNK  # 8 chunks per t-plane
CB = 8  # chunks per super-chunk
NSC = NCHUNK // CB  # 32

_NC_CACHE = {}


def build_nc():
    import concourse.bacc as bacc
    import concourse.bass as bass
    import concourse.mybir as mybir
    import concourse.tile as tile
    from concourse.masks import make_identity

    fp32 = mybir.dt.float32
    bf16 = mybir.dt.bfloat16
    AF = mybir.ActivationFunctionType

    nc = bacc.Bacc(None, target_bir_lowering=False)

    x_d = nc.declare_dram_parameter("x", [C, THW], fp32, isOutput=False)
    wt_d = nc.declare_dram_parameter("wt_cat", [C, 3 * M], bf16, isOutput=False)
    w1t_d = nc.declare_dram_parameter("w1t", [CS, CS], fp32, isOutput=False)
    b1_d = nc.declare_dram_parameter("b1", [CS, 1], fp32, isOutput=False)
    w2t_d = nc.declare_dram_parameter("w2t", [CS, H * W], fp32, isOutput=False)
    b2pt_d = nc.declare_dram_parameter("b2pt", [FCHUNK, HWBLK], fp32, isOutput=False)
    cond_d = nc.declare_dram_parameter("cond", [CS, 1], fp32, isOutput=False)
    ebr_d = nc.declare_dram_parameter("ebrho_b", [FCHUNK, N], bf16, isOutput=False)
    bphi_d = nc.declare_dram_parameter("bphi_b", [N, M], fp32, isOutput=False)
    ebrc_d = nc.declare_dram_parameter("ebrho_col", [N, 1], fp32, isOutput=False)
    oz_d = nc.declare_dram_parameter("out_z", [M, THW], bf16, isOutput=True)
    oa_d = nc.declare_dram_parameter("out_attn", [N, THW], bf16, isOutput=True)
    orw_d = nc.declare_dram_parameter("out_rbw", [N, 1], fp32, isOutput=True)

    with tile.TileContext(nc) as tc:
        import contextlib

        ctx = contextlib.ExitStack()
        with ctx:
            singles = ctx.enter_context(tc.tile_pool(name="singles", bufs=1))
            xpool = ctx.enter_context(tc.tile_pool(name="xp", bufs=3))
            work = ctx.enter_context(tc.tile_pool(name="work", bufs=4))
            psum_a = ctx.enter_context(tc.tile_pool(name="ps_a", bufs=2, space="PSUM"))
            psum_bv = ctx.enter_context(tc.tile_pool(name="ps_bv", bufs=1, space="PSUM"))
            psum_t = ctx.enter_context(tc.tile_pool(name="ps_t", bufs=3, space="PSUM"))
            psum_u = ctx.enter_context(tc.tile_pool(name="ps_u", bufs=1, space="PSUM"))

            ident = singles.tile([FCHUNK, FCHUNK], bf16)
            make_identity(nc, ident)

            wt_sb = singles.tile([C, 3 * M], bf16)
            nc.sync.dma_start(out=wt_sb, in_=wt_d[:, :])
            w1t_sb = singles.tile([CS, CS], fp32)
            nc.sync.dma_start(out=w1t_sb, in_=w1t_d[:, :])
            b1_sb = singles.tile([CS, 1], fp32)
            nc.sync.dma_start(out=b1_sb, in_=b1_d[:, :])
            w2t_sb = singles.tile([CS, H * W], fp32)
            nc.sync.dma_start(out=w2t_sb, in_=w2t_d[:, :])
            b2pt_sb = singles.tile([FCHUNK, HWBLK], fp32)
            nc.sync.dma_start(out=b2pt_sb, in_=b2pt_d[:, :])
            cond_sb = singles.tile([CS, 1], fp32)
            nc.sync.dma_start(out=cond_sb, in_=cond_d[:, :])
            ebr_sb = singles.tile([FCHUNK, 1, N], bf16)
            nc.sync.dma_start(out=ebr_sb, in_=ebr_d[:, :].rearrange("p (o n) -> p o n", o=1))
            bphi_sb = singles.tile([N, M], fp32)
            nc.sync.dma_start(out=bphi_sb, in_=bphi_d[:, :])
            ebrc_sb = singles.tile([N, 1], fp32)
            nc.sync.dma_start(out=ebrc_sb, in_=ebrc_d[:, :])

            ones_col = singles.tile([C, 1], fp32)
            nc.vector.memset(ones_col, 1.0)
            ones_row = singles.tile([1, C], fp32)
            nc.vector.memset(ones_row, 1.0)

            # ---------------- tiny FC-cond (fp32, negligible) ----------------
            ps_h1 = psum_t.tile([CS, 1], fp32, tag="pt")
            nc.tensor.matmul(ps_h1, w1t_sb, cond_sb)
            h1_sb = work.tile([CS, 1], fp32)
            nc.scalar.activation(h1_sb, ps_h1, AF.Relu, bias=b1_sb, scale=1.0)
            ps_h2 = psum_t.tile([FCHUNK, HWBLK], fp32, tag="pt")
            for k in range(HWBLK):
                nc.tensor.matmul(
                    ps_h2[:, k : k + 1],
                    w2t_sb[:, k * FCHUNK : (k + 1) * FCHUNK],
                    h1_sb,
                )
            h2_sb = work.tile([FCHUNK, HWBLK], fp32)
            nc.vector.tensor_add(h2_sb, ps_h2, b2pt_sb)
            nc.scalar.activation(h2_sb, h2_sb, AF.Relu)
            expc_sb = work.tile([FCHUNK, HWBLK], fp32)
            sc_sb = work.tile([FCHUNK, 1], fp32)
            nc.scalar.activation(expc_sb, h2_sb, AF.Exp, accum_out=sc_sb)
            ps_s = psum_t.tile([1, 1], fp32, tag="pt")
            nc.tensor.matmul(ps_s, ones_col, sc_sb)
            rc_sb = work.tile([1, 1], fp32)
            nc.vector.reciprocal(rc_sb, ps_s)
            ps_rc = psum_t.tile([FCHUNK, 1], fp32, tag="pt")
            nc.tensor.matmul(ps_rc, ones_row, rc_sb)
            rcp_sb = work.tile([FCHUNK, 1], fp32)
            nc.scalar.copy(rcp_sb, ps_rc)
            cm_sb = singles.tile([FCHUNK, HWBLK, 1], fp32)
            nc.vector.tensor_scalar_mul(
                cm_sb, in0=expc_sb.rearrange("p (k o) -> p k o", o=1), scalar1=rcp_sb
            )
            cmones_sb = singles.tile([FCHUNK, HWBLK, 2], bf16)
            nc.vector.memset(cmones_sb, 1.0)
            nc.vector.tensor_copy(cmones_sb[:, :, 0:1], cm_sb)

            # ---------------- x load + bf16 convert ----------------
            XPIECE = 2048
            NXD = THW // XPIECE  # 8
            xbf = singles.tile([C, THW], bf16)
            for i in range(NXD):
                s = i * XPIECE
                xt = xpool.tile([C, XPIECE], fp32, tag="xt")
                nc.sync.dma_start(out=xt, in_=x_d[:, s : s + XPIECE])
                nc.vector.tensor_copy(xbf[:, s : s + XPIECE], xt)

            # ---------------- main loop ----------------
            vq = singles.tile([2 * N, THW], bf16)  # rows 0:64 V, 64:128 attn raw
            ps_u_t = psum_u.tile([N, M + 2], fp32)

            for sc in range(NSC):
                c0 = sc * CB
                f0 = c0 * FCHUNK
                tidx = c0 // HWBLK
                hwb0 = c0 % HWBLK  # 0 or 4

                pcA = psum_a.tile([FCHUNK, CB, M], fp32, tag="pcA")
                pcBV = psum_bv.tile([FCHUNK, CB, 2 * M], fp32, tag="pcBV")
                for j in range(CB):
                    xsl = xbf[:, f0 + j * FCHUNK : f0 + (j + 1) * FCHUNK]
                    nc.tensor.matmul(pcA[:, j, :], xsl, wt_sb[:, 0:M])
                    nc.tensor.matmul(pcBV[:, j, :], xsl, wt_sb[:, M : 3 * M])

                ebv = work.tile([FCHUNK, CB, 2 * N], bf16, tag="ebv")
                nc.scalar.activation(ebv, pcBV, AF.Exp)
                eb = ebv[:, :, 0:N]
                ev = ebv[:, :, N : 2 * N]
                csum = work.tile([FCHUNK, CB], fp32, tag="csum")
                scr = work.tile([FCHUNK, CB, N], fp32, tag="scr")
                nc.vector.tensor_mul(scr, ev, ebr_sb.to_broadcast([FCHUNK, CB, N]))
                nc.vector.reduce_sum(csum, scr, axis=mybir.AxisListType.X)
                nc.vector.reciprocal(csum, csum)
                tbs = work.tile([FCHUNK, CB, 2 * N], bf16, tag="tbs")
                nc.vector.tensor_mul(
                    tbs[:, :, 0:N],
                    ev,
                    csum.rearrange("p (c o) -> p c o", o=1).to_broadcast(
                        [FCHUNK, CB, N]
                    ),
                )
                nc.vector.tensor_mul(tbs[:, :, N : 2 * N], eb, tbs[:, :, 0:N])

                pt = psum_t.tile([FCHUNK, CB, FCHUNK], bf16, tag="pt")
                for j in range(CB):
                    nc.tensor.transpose(pt[:, j, :], tbs[:, j, :], ident)

                nc.scalar.copy(vq[:, f0 : f0 + CB * FCHUNK], pt)
                nc.sync.dma_start(
                    out=oa_d[:, f0 : f0 + CB * FCHUNK],
                    in_=vq[N : 2 * N, f0 : f0 + CB * FCHUNK],
                )

                urhs = work.tile([FCHUNK, CB, M + 2], bf16, tag="urhs")
                nc.vector.tensor_mul(
                    urhs[:, :, 0:M],
                    pcA,
                    cm_sb[:, hwb0 : hwb0 + CB, :].to_broadcast([FCHUNK, CB, M]),
                )
                nc.vector.tensor_copy(
                    urhs[:, :, M : M + 2], cmones_sb[:, hwb0 : hwb0 + CB, :]
                )
                for j in range(CB):
                    nc.tensor.matmul(
                        ps_u_t,
                        ebv[:, j, 0:N],
                        urhs[:, j, :],
                        start=(sc == 0 and j == 0),
                        stop=(sc == NSC - 1 and j == CB - 1),
                    )

            # ---------------- post loop ----------------
            u_sb = work.tile([N, M + 2], fp32)
            nc.scalar.copy(u_sb, ps_u_t)
            rb_sb = work.tile([N, 1], fp32)
            nc.vector.reciprocal(rb_sb, u_sb[:, M + 1 : M + 2])
            corr = work.tile([N, M], fp32)
            nc.vector.tensor_scalar_mul(corr, in0=bphi_sb, scalar1=u_sb[:, M : M + 1])
            nc.vector.tensor_add(corr, corr, u_sb[:, 0:M])
            rbw_sb = work.tile([N, 1], fp32)
            nc.vector.tensor_mul(rbw_sb, rb_sb, ebrc_sb)
            abt_sb = work.tile([N, M], bf16)
            nc.vector.tensor_scalar_mul(abt_sb, in0=corr, scalar1=rbw_sb)

            ZBLK = 512
            ZGRP = 4
            for bg in range(THW // (ZBLK * ZGRP)):
                zt = work.tile([M, ZGRP, ZBLK], bf16, tag="zt")
                for k in range(ZGRP):
                    bi = bg * ZGRP + k
                    s = bi * ZBLK
                    pz = psum_t.tile([M, ZBLK], fp32, tag="pt")
                    nc.tensor.matmul(pz, abt_sb, vq[0:N, s : s + ZBLK])
                    if k % 4 == 3:
                        nc.vector.tensor_copy(zt[:, k, :], pz)
                    else:
                        nc.scalar.copy(zt[:, k, :], pz)
                nc.sync.dma_start(
                    out=oz_d[:, bg * ZGRP * ZBLK : (bg + 1) * ZGRP * ZBLK], in_=zt
                )

            nc.sync.dma_start(out=orw_d[:, :], in_=rbw_sb)


    nc.compile()
    return nc


def _prep_shared(W_phi, b_phi, W_theta, b_theta, W_rho, b_rho, W1, b1, W2, b2):
    import ml_dtypes

    f32 = np.float32
    bf = ml_dtypes.bfloat16
    wt_cat = np.concatenate(
        [np.asarray(W_phi).T, np.asarray(W_theta).T, np.asarray(W_rho).T], axis=1
    ).astype(f32)
    shared = {
        "wt_cat": np.ascontiguousarray(wt_cat.astype(bf)),
        "w1t": np.ascontiguousarray(np.asarray(W1).T.astype(f32)),
        "b1": np.asarray(b1).astype(f32).reshape(CS, 1),
        "w2t": np.ascontiguousarray(np.asarray(W2).T.astype(f32)),
        "b2pt": np.ascontiguousarray(
            np.asarray(b2).astype(f32).reshape(HWBLK, FCHUNK).T
        ),
        "ebrho_b": np.ascontiguousarray(
            np.tile(np.exp(np.asarray(b_rho).astype(f32))[None, :], (FCHUNK, 1)).astype(bf)
        ),
        "bphi_b": np.ascontiguousarray(
            np.tile(np.asarray(b_phi).astype(f32)[None, :], (N, 1))
        ),
        "ebrho_col": np.ascontiguousarray(
            np.exp(np.asarray(b_rho).astype(f32)).reshape(N, 1)
        ),
    }
    return shared


def _install_ntff_hook():
    """The container's antenv stub lacks axon_hooks; inject it and register
    the ctypes NTFF profiling hook so trace=True works under axon."""
    import sys
    import types

    if "antenv.axon_hooks" in sys.modules:
        return
    import antenv

    mod = types.ModuleType("antenv.axon_hooks")
    _state = {}
    mod.set_axon_ntff_profile_hook = lambda h: _state.__setitem__("h", h)
    mod.get_axon_ntff_profile_hook = lambda: _state.get("h")
    sys.modules["antenv.axon_hooks"] = mod
    antenv.axon_hooks = mod
    try:
        from trn_agent_boot.trn_boot import _ntff_profile_via_ctypes

        hook = _ntff_profile_via_ctypes("/opt/axon/libaxon_pjrt.so")
        if hook is not None:
            mod.set_axon_ntff_profile_hook(hook)
    except Exception as e:  # profiling degrades, run still works
        print("ntff hook install failed:", e)


def run_cores(inputs, trace=False):
    from concourse.bass_utils import run_bass_kernel_spmd

    if trace:
        _install_ntff_hook()

    if "nc" not in _NC_CACHE:
        _NC_CACHE["nc"] = build_nc()
    nc = _NC_CACHE["nc"]

    x_full = np.asarray(inputs["input"], dtype=np.float32).reshape(B, C, THW)
    cond_full = np.asarray(inputs["condition"], dtype=np.float32)
    shared = _prep_shared(
        inputs["W_phi"], inputs["b_phi"], inputs["W_theta"], inputs["b_theta"],
        inputs["W_rho"], inputs["b_rho"], inputs["W1"], inputs["b1"],
        inputs["W2"], inputs["b2"],
    )
    in_maps = []
    for b in range(NCORES):
        m = dict(shared)
        m["x"] = np.ascontiguousarray(x_full[b])
        m["cond"] = np.ascontiguousarray(cond_full[b].reshape(CS, 1))
        in_maps.append(m)

    kw = {}
    if trace:
        kw = dict(trace=True, trace_cores=[0])
    res = run_bass_kernel_spmd(nc, in_maps, core_ids=list(range(NCORES)), **kw)
    return res


def kernel(**inputs):
    res = run_cores(inputs, trace=False)
    z = np.stack(
        [np.asarray(r["out_z"], dtype=np.float32) for r in res.results], axis=0
    ).reshape(B, M, T, H, W)
    # out_attn is raw eb*V in [n, (t, hw)] order; scale by rbw and permute on host
    attn = np.empty((B, N, H, W, T), dtype=np.float32)
    for b in range(B):
        a = np.asarray(res.results[b]["out_attn"], dtype=np.float32)
        rw = np.asarray(res.results[b]["out_rbw"], dtype=np.float32)
        a = (a * rw).reshape(N, T, H, W)
        attn[b] = a.transpose(0, 2, 3, 1)
    return z, attn
